# revision 40
# baseline (speedup 1.0000x reference)
"""Multi-head attention block (16 query heads, shared single K/V head) on
8 Trainium2 NeuronCores.

Reference computation (B=2, S=2048, D=2048, HQ=16, DH=128, fp32):
    q = (x @ Wq + bq)  -> [B, S, 16, 128]
    k = x @ Wk + bk    -> [B, S, 128]   (single shared K/V head)
    v = x @ Wv + bv    -> [B, S, 128]
    attn = softmax(q k^T / sqrt(128))
    out = (attn @ v) reshaped -> [B, S, D];  y = out @ Wo + bo

Sharding: batch x sequence-block data parallel. Core c handles batch c//4,
query rows (c%4)*512 .. +512, for ALL 16 heads. No inter-core collectives;
every core emits a disjoint slab of the final output.

All matmuls run in bfloat16 (fp32 accumulation in PSUM). Schedule (v2,
fused pipeline):

  warm : 16 dummy matmuls on memset tiles (no DMA deps) keep the PE busy
         while the first DMAs land, so HAM un-throttles (1.2->2.4 GHz)
         before real work.
  A    : k/v projections over the full sequence, d-chunk streamed from HBM
         (DMA-paced; xT chunks keep queue priority, xq/wq tiles slot in
         late). All biases are folded into the matmuls as rank-1
         [1,x] @ [1,N] updates so the PSUM->SBUF drains are pure copies,
         split across ScalarE and VectorE. The drain whose bank the next
         phase needs goes first.
  boot : q-projection for heads 0-1 + PE transposes of v into [key,dh]
         layout + bias broadcast for the output projection.
  B1'  : fused per-head attention x q-projection pipeline, 8 steps per
         head. Each step: 2 scores MMs (PE) -> exp (ScalarE) -> 2 p@v MMs
         (PE, 2 steps later) + 2 q-projection MMs for head h+2 (PE) +
         bf16 tree accumulation of the softmax denominator (VectorE).
         Per head: one [128,1]-ones matmul reduces the tree root over
         partitions (+3-step skew), reciprocal (VectorE) and a partition
         broadcast on the otherwise-idle GpSimd engine, then normalize
         (+3 more steps of skew so the PE never waits on the round-trip).
         ScalarE also drains each folded head's q tile (pure copy).
  C    : output projection y = out @ Wo + bo with Wo prefetched to SBUF
         during B1'. First chain overlaps B1's tail (its PSUM banks are
         the score banks the last exp freed).

Measured: 303-306us on HW (baseline schedule: 360us), rel err 5.36e-3.
Schedules tried and rejected: pv-before-scores emission (+5us: head-of-
line blocking on the 2-step-old exp), front-loading head 2's fold into
the first steps (neutral), early xq/wq DMA placement (starves the xT
stream mid-phase-A and re-throttles HAM).

Final measured: 295.3-298.4us across 5 clean runs (baseline: 360.3us),
rel err 5.36e-3.  NOTE: many back-to-back runs downclock the whole chip
1.2x (MM gap 215->258ns, exp 1114->1336ns, HAM still K=8/8); ~5min idle
recovers.  NEXT LEAD (+~6us, one careful run): the per-head fold bias
matmul (rank-1, 1-partition lhsT) causes two stalls per head (~310ns +
~545ns, x14 heads).  Fix = full-K bias: biases pre-divided by 128
(exact), replicated to [128,x] bf16, bias MM = lhsT[128,128] x
ones[128,N].  FAILED ATTEMPT (rel err 0.269 even with zero biases =
structural corruption): replicating via gpsimd.partition_broadcast on
BF16 [1,2048]; the proven broadcasts were FP32 [1,512] -- suspect the
ucode mishandles 2-byte dtype or the larger free size.  Safe recipe:
build the replicated tiles with PE broadcast matmuls (lhsT=ones_fr,
rhs=bias row as f32r, like bo_b) in the warm-up PSUM bank, DVE-copy to
bf16.  Also rejected: AllGather k/v dedup (72.6us cc-active per 1MB in
this harness); dma_start_transpose for v (neutral); warm-matmul fillers
in DMA-bound phase A (3 attempts, each +1..2us); fold-before-scores
order (neutral); psc bufs=3 (no PSUM left).
"""

import numpy as np
import ml_dtypes

B, S, D = 2, 2048, 2048
HQ, DH = 16, 128
SBLK = S // 4          # 512 query rows per core
N_CORES = 8
SCALE = 1.0 / float(np.sqrt(DH))

ND = D // 128          # 16 contraction chunks
NT = S // 128          # 16 key tiles
NQ = SBLK // 128       # 4 query row-tiles per core
NSH = NT // 2          # 8 pipeline steps per head

_cache = {}


def _round_fp32r(a):
    """Round fp32 to fp32r (1s+8e+11m) with round-to-nearest-even-ish."""
    b = np.ascontiguousarray(a, dtype=np.float32).view(np.uint32)
    bias = np.uint32(0x7FF) + ((b >> np.uint32(12)) & np.uint32(1))
    return ((b + bias) & np.uint32(0xFFFFF000)).view(np.float32)


def _build():
    from concourse import bacc, mybir, tile
    from concourse.masks import make_identity

    F32 = mybir.dt.float32
    F32R = mybir.dt.float32r
    BF16 = mybir.dt.bfloat16
    Exp = mybir.ActivationFunctionType.Exp
    Copy = mybir.ActivationFunctionType.Copy
    mult = mybir.AluOpType.mult
    add = mybir.AluOpType.add

    nc = bacc.Bacc("TRN2", target_bir_lowering=False, debug=False,
                   num_devices=N_CORES)

    xT = nc.dram_tensor("xT", [D, S], BF16, kind="ExternalInput").ap()
    xTq = nc.dram_tensor("xTq", [D, SBLK], BF16, kind="ExternalInput").ap()
    Wq = nc.dram_tensor("Wq", [D, D], BF16, kind="ExternalInput").ap()
    Wk = nc.dram_tensor("Wk", [D, DH], BF16, kind="ExternalInput").ap()
    Wv = nc.dram_tensor("Wv", [D, DH], BF16, kind="ExternalInput").ap()
    Wo = nc.dram_tensor("Wo", [D, D], BF16, kind="ExternalInput").ap()
    bqT_d = nc.dram_tensor("bqT", [1, D], F32R, kind="ExternalInput").ap()
    bkT_d = nc.dram_tensor("bkT", [1, DH], F32R, kind="ExternalInput").ap()
    bvT_d = nc.dram_tensor("bvT", [1, DH], F32R, kind="ExternalInput").ap()
    bo_d = nc.dram_tensor("bo", [1, D], F32R, kind="ExternalInput").ap()
    ones_d = nc.dram_tensor("ones", [128, 128], BF16, kind="ExternalInput").ap()
    ones_fd = nc.dram_tensor("onesf", [1, 128], F32R, kind="ExternalInput").ap()
    y = nc.dram_tensor("y", [SBLK, D], F32, kind="ExternalOutput").ap()

    with tile.TileContext(nc) as tc, nc.allow_low_precision(
        reason="bf16 matmul pipeline; verified against fp32 reference"
    ):
        with (
            tc.tile_pool(name="const", bufs=1) as cpool,
            tc.tile_pool(name="live", bufs=1) as lpool,      # kT, vT, v_nat, xq, qT
            tc.tile_pool(name="ot", bufs=HQ) as otpool,      # 16 head outputs
            tc.tile_pool(name="wq", bufs=4) as wqpool,       # Wq stream
            tc.tile_pool(name="pt", bufs=4) as ptpool,       # exp outputs
            tc.tile_pool(name="den", bufs=2) as dpool,       # denominator tree
            tc.tile_pool(name="rc", bufs=2) as rcpool,       # recip + broadcast
            tc.tile_pool(name="yp", bufs=3) as ypool,
        ):
            # ---- constants -------------------------------------------------
            ones = cpool.tile([128, 128], BF16)
            nc.sync.dma_start(out=ones[:, :], in_=ones_d[:, :])
            ones_col = ones[:, 0:1]
            ones_fr = cpool.tile([1, 128], F32R)
            nc.sync.dma_start(out=ones_fr[:, :], in_=ones_fd[:, :])
            bkT = cpool.tile([1, DH], F32R)
            nc.sync.dma_start(out=bkT[:, :], in_=bkT_d[:, :])
            bvT = cpool.tile([1, DH], F32R)
            nc.sync.dma_start(out=bvT[:, :], in_=bvT_d[:, :])
            bqT = cpool.tile([1, D], F32R)
            nc.sync.dma_start(out=bqT[:, :], in_=bqT_d[:, :])
            bo_row = cpool.tile([1, D], F32R)
            nc.sync.dma_start(out=bo_row[:, :], in_=bo_d[:, :])

            ones_row = cpool.tile([1, SBLK], BF16)
            nc.vector.memset(ones_row[:, :], 1.0)
            ones_big = cpool.tile([128, SBLK], BF16)
            nc.vector.memset(ones_big[:, :], 1.0)
            # biases replicated across partitions (values arrive /128, exact)
            # so bias matmuls use a full 128x128 stationary operand -- the
            # rank-1 1-partition lhsT broke PE weight-buffer pipelining
            # (~310+545ns of stalls per folded head).  Built below with PE
            # broadcast matmuls (the proven bo_b pattern; GpSimd
            # partition_broadcast corrupts bf16/[1,2048] operands).
            bqb = cpool.tile([128, D], BF16)
            bkb = cpool.tile([128, DH], BF16)
            bvb = cpool.tile([128, DH], BF16)
            warm_rhs = cpool.tile([128, SBLK], BF16)
            nc.vector.memset(warm_rhs[:, :], 0.0)
            warm_lhs = cpool.tile([128, 128], BF16)
            nc.vector.memset(warm_lhs[:, :], 0.0)
            ident = cpool.tile([128, 128], BF16)
            make_identity(nc, ident[:, :])
            bo_b = cpool.tile([128, D], F32)

            warm = cpool.tile([1, 1], BF16)
            nc.scalar.activation(warm[:, :], ones[0:1, 0:1], Exp, scale=1.0)

            kT = lpool.tile([128, S], BF16)
            vT = lpool.tile([128, S], BF16)
            v_nat = lpool.tile([128, NT, DH], BF16)
            xq = lpool.tile([128, ND, SBLK], BF16)
            qT_all = lpool.tile([128, HQ, SBLK], BF16)

            # q-projection PSUM bank allocated BEFORE phase A's pool so the
            # bootstrap q-projections never wait on the pool-wide barrier
            # against phase A's eight PSUM drains.
            pqp = tc.alloc_tile_pool(name="pq", bufs=1, space="PSUM")

            def q_proj(h, wq_t):
                pq = pqp.tile([128, SBLK], F32, tag="pq", name=f"pq{h}")
                for d in range(ND):
                    nc.tensor.matmul(
                        pq[:, :], lhsT=wq_t[:, d, :], rhs=xq[:, d, :],
                        start=(d == 0), stop=False,
                    )
                nc.tensor.matmul(
                    pq[:, :], lhsT=bqb[:, h * 128:(h + 1) * 128],
                    rhs=ones_big[:, :], start=False, stop=True,
                )
                nc.scalar.activation(qT_all[:, h, :], pq[:, :], Copy)

            # ---- phase A: k/v projections over the full sequence -----------
            with tc.tile_pool(name="pha", bufs=1) as apool:
                wk_all = apool.tile([128, ND, DH], BF16)
                nc.sync.dma_start(
                    out=wk_all[:, :, :],
                    in_=Wk.rearrange("(n p) d -> p n d", p=128),
                )
                wv_all = apool.tile([128, ND, DH], BF16)
                nc.sync.dma_start(
                    out=wv_all[:, :, :],
                    in_=Wv.rearrange("(n p) d -> p n d", p=128),
                )
                # xT chunks stream in consumption order; xq and the first
                # q-weight tiles are interleaved late enough not to delay the
                # matmul stream (PE has buffered chunks by then) but early
                # enough to be resident when the bootstrap needs them.
                wq_tiles = {}

                def wq_dma(h):
                    wq_tiles[h] = wqpool.tile([128, ND, 128], BF16, tag="wq",
                                              name=f"wq{h}")
                    nc.sync.dma_start(
                        out=wq_tiles[h][:, :, :],
                        in_=Wq[:, h * 128:(h + 1) * 128].rearrange(
                            "(n p) m -> p n m", p=128
                        ),
                    )

                # xT chunks keep DMA-queue priority (they pace phase A); the
                # xq quarters and first wq tiles slot in late, where the PE
                # has buffered chunks, and land just before the bootstrap
                # needs them.
                xT_all = apool.tile([128, ND, S], BF16)
                xq_quarter = {9: 0, 10: 1, 12: 2, 13: 3}
                for d in range(ND):
                    nc.sync.dma_start(
                        out=xT_all[:, d, :], in_=xT[d * 128:(d + 1) * 128, :]
                    )
                    if d in xq_quarter:
                        q = xq_quarter[d]
                        nc.sync.dma_start(
                            out=xq[:, 4 * q:4 * (q + 1), :],
                            in_=xTq[512 * q:512 * (q + 1), :].rearrange(
                                "(n p) s -> p n s", p=128
                            ),
                        )
                    if d == 11:
                        wq_dma(0)
                    elif d == 15:
                        wq_dma(1)
                        wq_dma(2)

                with tc.tile_pool(name="pacc", bufs=1, space="PSUM") as pacc:
                    psum_k = pacc.tile([128, S], F32, tag="pk")
                    psum_v = pacc.tile([128, 1536], F32, tag="pv")

                    # PE warm-up in the (pre-allocated) q bank: keep the
                    # array busy while DMAs land so the HAM clock gate opens
                    # before the real stream begins.  No DMA dependencies.
                    warm_t = pqp.tile([128, SBLK], F32, tag="pq",
                                      name="warmpq")
                    for w in range(16):
                        nc.tensor.matmul(
                            warm_t[:, :],
                            lhsT=warm_lhs[:, :], rhs=warm_rhs[:, :],
                            start=True, stop=True,
                        )
                    # replicate biases across partitions (DMA-bound window,
                    # the PE/DVE round-trips here are free)
                    for j in range(D // 512):
                        nc.tensor.matmul(
                            warm_t[:, :], lhsT=ones_fr[0:1, :],
                            rhs=bqT[0:1, j * 512:(j + 1) * 512],
                            start=True, stop=True,
                        )
                        nc.vector.tensor_copy(
                            bqb[:, j * 512:(j + 1) * 512], warm_t[:, :]
                        )
                    nc.tensor.matmul(
                        warm_t[:, 0:DH], lhsT=ones_fr[0:1, :],
                        rhs=bkT[0:1, :], start=True, stop=True,
                    )
                    nc.vector.tensor_copy(bkb[:, :], warm_t[:, 0:DH])
                    nc.tensor.matmul(
                        warm_t[:, 0:DH], lhsT=ones_fr[0:1, :],
                        rhs=bvT[0:1, :], start=True, stop=True,
                    )
                    nc.vector.tensor_copy(bvb[:, :], warm_t[:, 0:DH])

                    for d in range(ND):
                        for nb in range(4):
                            sl = slice(nb * 512, (nb + 1) * 512)
                            nc.tensor.matmul(
                                psum_k[:, sl],
                                lhsT=wk_all[:, d, :],
                                rhs=xT_all[:, d, sl],
                                start=(d == 0), stop=False,
                            )
                        for nb in range(3):
                            sl = slice(nb * 512, (nb + 1) * 512)
                            nc.tensor.matmul(
                                psum_v[:, sl],
                                lhsT=wv_all[:, d, :],
                                rhs=xT_all[:, d, sl],
                                start=(d == 0), stop=False,
                            )
                        if d == 14:
                            # fill the DMA-starved tail of the chunk stream
                            q_proj(0, wq_tiles[0])
                    # rank-1 bias updates close each accumulation group
                    for nb in range(4):
                        sl = slice(nb * 512, (nb + 1) * 512)
                        nc.tensor.matmul(
                            psum_k[:, sl], lhsT=bkb[:, :],
                            rhs=ones_big[:, :], start=False, stop=True,
                        )
                    for nb in range(3):
                        sl = slice(nb * 512, (nb + 1) * 512)
                        nc.tensor.matmul(
                            psum_v[:, sl], lhsT=bvb[:, :],
                            rhs=ones_big[:, :], start=False, stop=True,
                        )
                    # drains split across DVE + ScalarE; kT block 0 first so
                    # its bank is free for v block 3's second pass below.
                    nc.vector.tensor_copy(kT[:, 0:512], psum_k[:, 0:512])
                    nc.scalar.activation(vT[:, 0:512], psum_v[:, 0:512], Copy)
                    nc.vector.tensor_copy(vT[:, 512:1024], psum_v[:, 512:1024])
                    nc.scalar.activation(vT[:, 1024:1536], psum_v[:, 1024:1536],
                                         Copy)
                    nc.vector.tensor_copy(kT[:, 512:1024], psum_k[:, 512:1024])
                    nc.scalar.activation(kT[:, 1024:1536], psum_k[:, 1024:1536],
                                         Copy)
                    nc.vector.tensor_copy(kT[:, 1536:2048], psum_k[:, 1536:2048])
                    # q1 covers the kT block-0 drain latency on the PE
                    q_proj(1, wq_tiles[1])
                    # v block 3 (keys 1536-2048): second pass through the
                    # freed k bank.  Pure PE work; the chunk data is SBUF
                    # resident, so this rides the DMA-bound region for free.
                    for d in range(ND):
                        nc.tensor.matmul(
                            psum_k[:, 0:512], lhsT=wv_all[:, d, :],
                            rhs=xT_all[:, d, 1536:2048],
                            start=(d == 0), stop=False,
                        )
                    nc.tensor.matmul(
                        psum_k[:, 0:512], lhsT=bvb[:, :],
                        rhs=ones_big[:, :], start=False, stop=True,
                    )
                    nc.scalar.activation(vT[:, 1536:2048], psum_k[:, 0:512],
                                         Copy)

            # ---- bootstrap: v transposes + bo broadcast --------------------
            with tc.tile_pool(name="ptr", bufs=1, space="PSUM") as ptrp:
                # copies split across VectorE/ScalarE: B1's first scores wait
                # on these PSUM banks, and a serial DVE backlog here was the
                # 3.5us stall (plus HAM re-throttle) at the B1 handoff.
                for t in range(NT):
                    ptr = ptrp.tile([128, 128], BF16, tag="tr", bufs=2)
                    nc.tensor.transpose(
                        ptr[:, :], vT[:, t * 128:(t + 1) * 128], ident[:, :]
                    )
                    if t % 2 == 0:
                        nc.vector.tensor_copy(v_nat[:, t, :], ptr[:, :])
                    else:
                        nc.scalar.activation(v_nat[:, t, :], ptr[:, :], Copy)
                # bias broadcast for phase C: bo_b = ones(128) x bo_row
                for nb in range(D // 512):
                    sl = slice(nb * 512, (nb + 1) * 512)
                    pbo = ptrp.tile([128, 512], F32, tag="bo", bufs=2)
                    nc.tensor.matmul(
                        pbo[:, :], lhsT=ones_fr[0:1, :], rhs=bo_row[0:1, sl],
                        start=True, stop=True,
                    )
                    if nb % 2 == 0:
                        nc.scalar.activation(bo_b[:, sl], pbo[:, :], Copy)
                    else:
                        nc.vector.tensor_copy(bo_b[:, sl], pbo[:, :])

            # ---- phase B1': fused attention + q-projection pipeline --------
            outT_list = [None] * HQ
            NS = HQ * NSH
            with (
                tc.tile_pool(name="psc", bufs=2, space="PSUM") as pscp,
                tc.tile_pool(name="po", bufs=2, space="PSUM") as pop,
                tc.tile_pool(name="aux", bufs=1, space="PSUM") as auxp,
            ):
                pT_t, P_t, Bt_t, Ct_t, root_t = {}, {}, {}, {}, {}
                po_t, rc_t = {}, {}

                def emit_scores(s):
                    h, tp = divmod(s, NSH)
                    psc = pscp.tile([128, 2 * SBLK], F32, tag="sc")
                    for half in range(2):
                        t = tp * 2 + half
                        nc.tensor.matmul(
                            psc[:, half * SBLK:(half + 1) * SBLK],
                            lhsT=kT[:, t * 128:(t + 1) * 128],
                            rhs=qT_all[:, h, :],
                            start=True, stop=True,
                        )
                    return psc

                def emit_exp(s, psc):
                    h, tp = divmod(s, NSH)
                    pT = ptpool.tile([128, 2 * SBLK], BF16, tag="pT")
                    nc.scalar.activation(pT[:, :], psc[:, :], Exp, scale=SCALE)
                    pT_t[s] = pT
                    # denominator: pairwise add + bf16 tree on DVE
                    P = dpool.tile([128, SBLK], BF16, tag="P", name=f"P{tp}")
                    nc.vector.tensor_tensor(
                        P[:, :], pT[:, 0:SBLK], pT[:, SBLK:2 * SBLK], add
                    )
                    P_t[tp] = P
                    if tp % 2 == 1:
                        Bt = dpool.tile([128, SBLK], BF16, tag="B",
                                        name=f"B{tp // 2}")
                        nc.vector.tensor_tensor(
                            Bt[:, :], P_t.pop(tp - 1)[:, :], P_t.pop(tp)[:, :],
                            add,
                        )
                        Bt_t[tp // 2] = Bt
                    if tp in (3, 7):
                        Ct = dpool.tile([128, SBLK], BF16, tag="C",
                                        name=f"C{tp // 4}")
                        nc.vector.tensor_tensor(
                            Ct[:, :], Bt_t.pop(tp // 2 - 1)[:, :],
                            Bt_t.pop(tp // 2)[:, :], add,
                        )
                        Ct_t[tp // 4] = Ct
                    if tp == 7:
                        root = dpool.tile([128, SBLK], BF16, tag="root",
                                          name=f"root{h}")
                        nc.vector.tensor_tensor(
                            root[:, :], Ct_t.pop(0)[:, :], Ct_t.pop(1)[:, :],
                            add,
                        )
                        root_t[h] = root

                def emit_pv(s):
                    h, tp = divmod(s, NSH)
                    if tp == 0:
                        po_t[h] = pop.tile([128, SBLK], F32, tag="po",
                                           name=f"po{h}")
                    pT = pT_t.pop(s)
                    for half in range(2):
                        t = tp * 2 + half
                        nc.tensor.matmul(
                            po_t[h][:, :],
                            lhsT=v_nat[:, t, :],
                            rhs=pT[:, half * SBLK:(half + 1) * SBLK],
                            start=(t == 0), stop=(t == NT - 1),
                        )

                def emit_tail1(h):
                    # partition-reduce the tree root + reciprocal.  Runs 3
                    # steps after the head's last scores so the DVE tree root
                    # is long done when the PE matmul wants it.
                    # lhsT is the full 128x128 ones tile: every output
                    # partition gets the same column sum, i.e. the reduce IS
                    # the broadcast, at the same N=512 matmul cost — and the
                    # 128-col LDWEIGHTS pipelines like every other weight
                    # load (a 1-col load breaks the weight-buffer rhythm).
                    pd = auxp.tile([128, SBLK], F32, tag="aux", name=f"pd{h}")
                    nc.tensor.matmul(
                        pd[:, :], lhsT=ones[:, :], rhs=root_t.pop(h)[:, :],
                        start=True, stop=True,
                    )
                    rb = rcpool.tile([128, SBLK], F32, tag="rb", name=f"rb{h}")
                    nc.vector.reciprocal_approx_fast(rb[:, :], pd[:, :])
                    rc_t[h] = rb

                def emit_tail2(h):
                    # normalize; 3 steps after tail1 so the PE never waits on
                    # the DVE->GpSimd round-trip.
                    outT = otpool.tile([128, SBLK], BF16, tag="ot",
                                       name=f"ot{h}")
                    nc.vector.tensor_tensor(
                        outT[:, :], po_t.pop(h)[:, :], rc_t.pop(h)[:, :], mult
                    )
                    outT_list[h] = outT

                def emit_fold(s):
                    # q-projection for head h+2, 2 matmuls per step
                    h, tp = divmod(s, NSH)
                    hf = h + 2
                    if hf >= HQ:
                        return
                    if tp == 0:
                        # queue DMA for the next folded head's weights
                        hn = hf + 1
                        if hn < HQ:
                            wq_tiles[hn] = wqpool.tile(
                                [128, ND, 128], BF16, tag="wq", name=f"wq{hn}"
                            )
                            nc.sync.dma_start(
                                out=wq_tiles[hn][:, :, :],
                                in_=Wq[:, hn * 128:(hn + 1) * 128].rearrange(
                                    "(n p) m -> p n m", p=128
                                ),
                            )
                        # stream this head's slice of Wo for phase C
                        if h < 8:
                            for hh in range(2):
                                hw = h * 2 + hh
                                for db in range(D // 512):
                                    dsl = slice(db * 512, (db + 1) * 512)
                                    wt = wopool.tile(
                                        [128, 512], BF16, tag="wo",
                                        name=f"wo{db}_{hw}"
                                    )
                                    nc.sync.dma_start(
                                        out=wt[:, :],
                                        in_=Wo[hw * 128:(hw + 1) * 128, dsl],
                                    )
                                    wo_tiles[db, hw] = wt
                        fold_pq[hf] = pqp.tile([128, SBLK], F32, tag="pq",
                                               name=f"pqf{hf}")
                    pq = fold_pq[hf]
                    for dd in range(2):
                        d = tp * 2 + dd
                        nc.tensor.matmul(
                            pq[:, :], lhsT=wq_tiles[hf][:, d, :],
                            rhs=xq[:, d, :], start=(d == 0), stop=False,
                        )
                    if tp == NSH - 1:
                        nc.tensor.matmul(
                            pq[:, :], lhsT=bqb[:, hf * 128:(hf + 1) * 128],
                            rhs=ones_big[:, :], start=False, stop=True,
                        )
                        nc.scalar.activation(qT_all[:, hf, :], pq[:, :], Copy)

                # Wo prefetch pool opens after phase-A SBUF is released
                wopool = tc.alloc_tile_pool(name="wo", bufs=64)
                wo_tiles = {}
                fold_pq = {}

                psc_t = {}
                for s in range(NS + 6):
                    if s < NS:
                        psc_t[s] = emit_scores(s)
                        emit_fold(s)
                    if s == 2:
                        # dependency-free filler: the pipeline has no p@v
                        # work yet and scores(2) waits on exp(0), so the PE
                        # would idle here (and HAM would re-throttle the
                        # clock).  Aux bank is unused until head 0's tail.
                        wa = auxp.tile([128, SBLK], F32, tag="aux",
                                       name="warmaux")
                        for w in range(6):
                            nc.tensor.matmul(
                                wa[:, :], lhsT=warm_lhs[:, :],
                                rhs=warm_rhs[:, :], start=True, stop=True,
                            )
                    if 0 <= s - 1 < NS:
                        emit_exp(s - 1, psc_t.pop(s - 1))
                    if 0 <= s - 2 < NS:
                        emit_pv(s - 2)
                    if 0 <= s - 3 < NS and (s - 3) % NSH == NSH - 1:
                        emit_tail1((s - 3) // NSH)
                    if 0 <= s - 6 < NS and (s - 6) % NSH == NSH - 1:
                        emit_tail2((s - 6) // NSH)

            # ---- phase C: output projection y = out @ Wo + bo --------------
            with tc.tile_pool(name="py", bufs=3, space="PSUM") as pyp:
                for db in range(D // 512):
                    dsl = slice(db * 512, (db + 1) * 512)
                    for st in range(NQ):
                        py = pyp.tile([128, 512], F32, tag="py")
                        for hh in range(HQ):
                            nc.tensor.matmul(
                                py[:, :],
                                lhsT=outT_list[hh][:, st * 128:(st + 1) * 128],
                                rhs=wo_tiles[db, hh][:, :],
                                start=(hh == 0), stop=(hh == HQ - 1),
                            )
                        y_sb = ypool.tile([128, 512], F32, tag="y")
                        nc.vector.tensor_tensor(
                            y_sb[:, :], py[:, :], bo_b[:, dsl], add
                        )
                        nc.sync.dma_start(
                            out=y[st * 128:(st + 1) * 128, dsl], in_=y_sb[:, :]
                        )

            wopool.release()
            pqp.release()

    nc.compile()
    return nc


def _get_nc():
    if "nc" not in _cache:
        _cache["nc"] = _build()
    return _cache["nc"]


def _prepare_in_maps(x, Wq, bq, Wk, bk, Wv, bv, Wo, bo):
    bf = ml_dtypes.bfloat16
    x = np.asarray(x, dtype=np.float32)
    bqT = (_round_fp32r(bq) / np.float32(128))[None, :]
    bkT = (_round_fp32r(bk) / np.float32(128))[None, :]
    bvT = (_round_fp32r(bv) / np.float32(128))[None, :]
    bo = _round_fp32r(bo)[None, :]
    Wq_b = np.asarray(Wq, np.float32).astype(bf)
    Wk_b = np.asarray(Wk, np.float32).astype(bf)
    Wv_b = np.asarray(Wv, np.float32).astype(bf)
    Wo_b = np.asarray(Wo, np.float32).astype(bf)
    ones = np.ones((128, 128), bf)
    onesf = np.ones((1, 128), np.float32)

    xT = [np.ascontiguousarray(x[g].T).astype(bf) for g in range(B)]
    in_maps = []
    for c in range(N_CORES):
        g, blk = divmod(c, 4)
        s0 = blk * SBLK
        in_maps.append({
            "xT": xT[g],
            "xTq": np.ascontiguousarray(xT[g][:, s0:s0 + SBLK]),
            "Wq": Wq_b, "Wk": Wk_b, "Wv": Wv_b, "Wo": Wo_b,
            "bqT": bqT, "bkT": bkT, "bvT": bvT, "bo": bo,
            "ones": ones, "onesf": onesf,
        })
    return in_maps


def _assemble(results):
    out = np.empty((B, S, D), dtype=np.float32)
    for c in range(N_CORES):
        g, blk = divmod(c, 4)
        out[g, blk * SBLK:(blk + 1) * SBLK, :] = results[c]["y"]
    return out


def kernel(x, Wq, bq, Wk, bk, Wv, bv, Wo, bo):
    from concourse.bass_utils import run_bass_kernel_spmd

    in_maps = _prepare_in_maps(x, Wq, bq, Wk, bk, Wv, bv, Wo, bo)
    nc = _get_nc()
    res = run_bass_kernel_spmd(nc, in_maps, core_ids=list(range(N_CORES)))
    return _assemble(res.results)


# revision 41
# speedup vs baseline: 1.0102x; 1.0102x over previous
"""Multi-head attention block (16 query heads, shared single K/V head) on
8 Trainium2 NeuronCores.

Reference computation (B=2, S=2048, D=2048, HQ=16, DH=128, fp32):
    q = (x @ Wq + bq)  -> [B, S, 16, 128]
    k = x @ Wk + bk    -> [B, S, 128]   (single shared K/V head)
    v = x @ Wv + bv    -> [B, S, 128]
    attn = softmax(q k^T / sqrt(128))
    out = (attn @ v) reshaped -> [B, S, D];  y = out @ Wo + bo

Sharding: batch x sequence-block data parallel. Core c handles batch c//4,
query rows (c%4)*512 .. +512, for ALL 16 heads. No inter-core collectives;
every core emits a disjoint slab of the final output.

All matmuls run in bfloat16 (fp32 accumulation in PSUM). Schedule (v2,
fused pipeline):

  warm : 16 dummy matmuls on memset tiles (no DMA deps) keep the PE busy
         while the first DMAs land, so HAM un-throttles (1.2->2.4 GHz)
         before real work.
  A    : k/v projections over the full sequence, d-chunk streamed from HBM
         (DMA-paced; xT chunks keep queue priority, xq/wq tiles slot in
         late). All biases are folded into the matmuls as rank-1
         [1,x] @ [1,N] updates so the PSUM->SBUF drains are pure copies,
         split across ScalarE and VectorE. The drain whose bank the next
         phase needs goes first.
  boot : q-projection for heads 0-1 + PE transposes of v into [key,dh]
         layout + bias broadcast for the output projection.
  B1'  : fused per-head attention x q-projection pipeline, 8 steps per
         head. Each step: 2 scores MMs (PE) -> exp (ScalarE) -> 2 p@v MMs
         (PE, 2 steps later) + 2 q-projection MMs for head h+2 (PE) +
         bf16 tree accumulation of the softmax denominator (VectorE).
         Per head: one [128,1]-ones matmul reduces the tree root over
         partitions (+3-step skew), reciprocal (VectorE) and a partition
         broadcast on the otherwise-idle GpSimd engine, then normalize
         (+3 more steps of skew so the PE never waits on the round-trip).
         ScalarE also drains each folded head's q tile (pure copy).
  C    : output projection y = out @ Wo + bo with Wo prefetched to SBUF
         during B1'. First chain overlaps B1's tail (its PSUM banks are
         the score banks the last exp freed).

Measured: 303-306us on HW (baseline schedule: 360us), rel err 5.36e-3.
Schedules tried and rejected: pv-before-scores emission (+5us: head-of-
line blocking on the 2-step-old exp), front-loading head 2's fold into
the first steps (neutral), early xq/wq DMA placement (starves the xT
stream mid-phase-A and re-throttles HAM).

Final measured: 295.3-298.4us across 5 clean runs (baseline: 360.3us),
rel err 5.36e-3.  NOTE: many back-to-back runs downclock the whole chip
1.2x (MM gap 215->258ns, exp 1114->1336ns, HAM still K=8/8); ~5min idle
recovers.  LANDED: full-K bias matmuls (biases arrive /128 in f32r --
exact -- and are replicated to [128,x] bf16 via PE broadcast matmuls
through the warm-up PSUM bank; the rank-1 1-partition lhsT version cost
two stalls per folded head).  B1' now 161.7us with 26 stalls >=300ns
(was 164.8us / 39); totals 297.4/298.2us, statistically tied with the
rank-1 version but strictly cleaner at the engine level.  WARNING: do
NOT replicate via gpsimd.partition_broadcast on BF16 [1,2048] -- it
corrupts memory (rel err 0.269 with zero biases); the proven gpsimd
broadcasts were FP32 [1,512].  Remaining per-head stall: one ~432ns gap
(pd reduce / ACT handoff), ~3.4us total.  Also rejected: AllGather k/v dedup (72.6us cc-active per 1MB in
this harness); dma_start_transpose for v (neutral); warm-matmul fillers
in DMA-bound phase A (3 attempts, each +1..2us); fold-before-scores
order (neutral); psc bufs=3 (no PSUM left).
"""

import numpy as np
import ml_dtypes

B, S, D = 2, 2048, 2048
HQ, DH = 16, 128
SBLK = S // 4          # 512 query rows per core
N_CORES = 8
SCALE = 1.0 / float(np.sqrt(DH))

ND = D // 128          # 16 contraction chunks
NT = S // 128          # 16 key tiles
NQ = SBLK // 128       # 4 query row-tiles per core
NSH = NT // 2          # 8 pipeline steps per head

_cache = {}


def _round_fp32r(a):
    """Round fp32 to fp32r (1s+8e+11m) with round-to-nearest-even-ish."""
    b = np.ascontiguousarray(a, dtype=np.float32).view(np.uint32)
    bias = np.uint32(0x7FF) + ((b >> np.uint32(12)) & np.uint32(1))
    return ((b + bias) & np.uint32(0xFFFFF000)).view(np.float32)


def _build():
    from concourse import bacc, mybir, tile
    from concourse.masks import make_identity

    F32 = mybir.dt.float32
    F32R = mybir.dt.float32r
    BF16 = mybir.dt.bfloat16
    Exp = mybir.ActivationFunctionType.Exp
    Copy = mybir.ActivationFunctionType.Copy
    mult = mybir.AluOpType.mult
    add = mybir.AluOpType.add

    nc = bacc.Bacc("TRN2", target_bir_lowering=False, debug=False,
                   num_devices=N_CORES)

    xT = nc.dram_tensor("xT", [D, S], BF16, kind="ExternalInput").ap()
    xTq = nc.dram_tensor("xTq", [D, SBLK], BF16, kind="ExternalInput").ap()
    Wq = nc.dram_tensor("Wq", [D, D], BF16, kind="ExternalInput").ap()
    Wk = nc.dram_tensor("Wk", [D, DH], BF16, kind="ExternalInput").ap()
    Wv = nc.dram_tensor("Wv", [D, DH], BF16, kind="ExternalInput").ap()
    Wo = nc.dram_tensor("Wo", [D, D], BF16, kind="ExternalInput").ap()
    bqT_d = nc.dram_tensor("bqT", [1, D], F32R, kind="ExternalInput").ap()
    bkT_d = nc.dram_tensor("bkT", [1, DH], F32R, kind="ExternalInput").ap()
    bvT_d = nc.dram_tensor("bvT", [1, DH], F32R, kind="ExternalInput").ap()
    bo_d = nc.dram_tensor("bo", [1, D], F32R, kind="ExternalInput").ap()
    ones_d = nc.dram_tensor("ones", [128, 128], BF16, kind="ExternalInput").ap()
    ones_fd = nc.dram_tensor("onesf", [1, 128], F32R, kind="ExternalInput").ap()
    y = nc.dram_tensor("y", [SBLK, D], F32, kind="ExternalOutput").ap()

    with tile.TileContext(nc) as tc, nc.allow_low_precision(
        reason="bf16 matmul pipeline; verified against fp32 reference"
    ):
        with (
            tc.tile_pool(name="const", bufs=1) as cpool,
            tc.tile_pool(name="live", bufs=1) as lpool,      # kT, vT, v_nat, xq, qT
            tc.tile_pool(name="ot", bufs=HQ) as otpool,      # 16 head outputs
            tc.tile_pool(name="wq", bufs=4) as wqpool,       # Wq stream
            tc.tile_pool(name="pt", bufs=4) as ptpool,       # exp outputs
            tc.tile_pool(name="den", bufs=2) as dpool,       # denominator tree
            tc.tile_pool(name="rc", bufs=2) as rcpool,       # recip + broadcast
            tc.tile_pool(name="yp", bufs=3) as ypool,
        ):
            # ---- constants -------------------------------------------------
            ones = cpool.tile([128, 128], BF16)
            nc.sync.dma_start(out=ones[:, :], in_=ones_d[:, :])
            ones_col = ones[:, 0:1]
            ones_fr = cpool.tile([1, 128], F32R)
            nc.sync.dma_start(out=ones_fr[:, :], in_=ones_fd[:, :])
            bkT = cpool.tile([1, DH], F32R)
            nc.sync.dma_start(out=bkT[:, :], in_=bkT_d[:, :])
            bvT = cpool.tile([1, DH], F32R)
            nc.sync.dma_start(out=bvT[:, :], in_=bvT_d[:, :])
            bqT = cpool.tile([1, D], F32R)
            nc.sync.dma_start(out=bqT[:, :], in_=bqT_d[:, :])
            bo_row = cpool.tile([1, D], F32R)
            nc.sync.dma_start(out=bo_row[:, :], in_=bo_d[:, :])

            ones_row = cpool.tile([1, SBLK], BF16)
            nc.vector.memset(ones_row[:, :], 1.0)
            ones_big = cpool.tile([128, SBLK], BF16)
            nc.vector.memset(ones_big[:, :], 1.0)
            # biases replicated across partitions (values arrive /128, exact)
            # so bias matmuls use a full 128x128 stationary operand -- the
            # rank-1 1-partition lhsT broke PE weight-buffer pipelining
            # (~310+545ns of stalls per folded head).  Built below with PE
            # broadcast matmuls (the proven bo_b pattern; GpSimd
            # partition_broadcast corrupts bf16/[1,2048] operands).
            bqb = cpool.tile([128, D], BF16)
            bkb = cpool.tile([128, DH], BF16)
            bvb = cpool.tile([128, DH], BF16)
            warm_rhs = cpool.tile([128, SBLK], BF16)
            nc.vector.memset(warm_rhs[:, :], 0.0)
            warm_lhs = cpool.tile([128, 128], BF16)
            nc.vector.memset(warm_lhs[:, :], 0.0)
            ident = cpool.tile([128, 128], BF16)
            make_identity(nc, ident[:, :])
            bo_b = cpool.tile([128, D], F32)

            warm = cpool.tile([1, 1], BF16)
            nc.scalar.activation(warm[:, :], ones[0:1, 0:1], Exp, scale=1.0)

            kT = lpool.tile([128, S], BF16)
            vT = lpool.tile([128, S], BF16)
            v_nat = lpool.tile([128, NT, DH], BF16)
            xq = lpool.tile([128, ND, SBLK], BF16)
            qT_all = lpool.tile([128, HQ, SBLK], BF16)

            # q-projection PSUM bank allocated BEFORE phase A's pool so the
            # bootstrap q-projections never wait on the pool-wide barrier
            # against phase A's eight PSUM drains.
            pqp = tc.alloc_tile_pool(name="pq", bufs=1, space="PSUM")

            def q_proj(h, wq_t):
                pq = pqp.tile([128, SBLK], F32, tag="pq", name=f"pq{h}")
                for d in range(ND):
                    nc.tensor.matmul(
                        pq[:, :], lhsT=wq_t[:, d, :], rhs=xq[:, d, :],
                        start=(d == 0), stop=False,
                    )
                nc.tensor.matmul(
                    pq[:, :], lhsT=bqb[:, h * 128:(h + 1) * 128],
                    rhs=ones_big[:, :], start=False, stop=True,
                )
                nc.scalar.activation(qT_all[:, h, :], pq[:, :], Copy)

            # ---- phase A: k/v projections over the full sequence -----------
            with tc.tile_pool(name="pha", bufs=1) as apool:
                wk_all = apool.tile([128, ND, DH], BF16)
                nc.sync.dma_start(
                    out=wk_all[:, :, :],
                    in_=Wk.rearrange("(n p) d -> p n d", p=128),
                )
                wv_all = apool.tile([128, ND, DH], BF16)
                nc.sync.dma_start(
                    out=wv_all[:, :, :],
                    in_=Wv.rearrange("(n p) d -> p n d", p=128),
                )
                # xT chunks stream in consumption order; xq and the first
                # q-weight tiles are interleaved late enough not to delay the
                # matmul stream (PE has buffered chunks by then) but early
                # enough to be resident when the bootstrap needs them.
                wq_tiles = {}

                def wq_dma(h):
                    wq_tiles[h] = wqpool.tile([128, ND, 128], BF16, tag="wq",
                                              name=f"wq{h}")
                    nc.sync.dma_start(
                        out=wq_tiles[h][:, :, :],
                        in_=Wq[:, h * 128:(h + 1) * 128].rearrange(
                            "(n p) m -> p n m", p=128
                        ),
                    )

                # xT chunks keep DMA-queue priority (they pace phase A); the
                # xq quarters and first wq tiles slot in late, where the PE
                # has buffered chunks, and land just before the bootstrap
                # needs them.
                xT_all = apool.tile([128, ND, S], BF16)
                xq_quarter = {9: 0, 10: 1, 12: 2, 13: 3}
                for d in range(ND):
                    nc.sync.dma_start(
                        out=xT_all[:, d, :], in_=xT[d * 128:(d + 1) * 128, :]
                    )
                    if d in xq_quarter:
                        q = xq_quarter[d]
                        nc.sync.dma_start(
                            out=xq[:, 4 * q:4 * (q + 1), :],
                            in_=xTq[512 * q:512 * (q + 1), :].rearrange(
                                "(n p) s -> p n s", p=128
                            ),
                        )
                    if d == 11:
                        wq_dma(0)
                    elif d == 15:
                        wq_dma(1)
                        wq_dma(2)

                with tc.tile_pool(name="pacc", bufs=1, space="PSUM") as pacc:
                    psum_k = pacc.tile([128, S], F32, tag="pk")
                    psum_v = pacc.tile([128, 1536], F32, tag="pv")

                    # PE warm-up in the (pre-allocated) q bank: keep the
                    # array busy while DMAs land so the HAM clock gate opens
                    # before the real stream begins.  No DMA dependencies.
                    warm_t = pqp.tile([128, SBLK], F32, tag="pq",
                                      name="warmpq")
                    for w in range(16):
                        nc.tensor.matmul(
                            warm_t[:, :],
                            lhsT=warm_lhs[:, :], rhs=warm_rhs[:, :],
                            start=True, stop=True,
                        )
                    # replicate biases across partitions (DMA-bound window,
                    # the PE/DVE round-trips here are free)
                    for j in range(D // 512):
                        nc.tensor.matmul(
                            warm_t[:, :], lhsT=ones_fr[0:1, :],
                            rhs=bqT[0:1, j * 512:(j + 1) * 512],
                            start=True, stop=True,
                        )
                        nc.vector.tensor_copy(
                            bqb[:, j * 512:(j + 1) * 512], warm_t[:, :]
                        )
                    nc.tensor.matmul(
                        warm_t[:, 0:DH], lhsT=ones_fr[0:1, :],
                        rhs=bkT[0:1, :], start=True, stop=True,
                    )
                    nc.vector.tensor_copy(bkb[:, :], warm_t[:, 0:DH])
                    nc.tensor.matmul(
                        warm_t[:, 0:DH], lhsT=ones_fr[0:1, :],
                        rhs=bvT[0:1, :], start=True, stop=True,
                    )
                    nc.vector.tensor_copy(bvb[:, :], warm_t[:, 0:DH])

                    for d in range(ND):
                        for nb in range(4):
                            sl = slice(nb * 512, (nb + 1) * 512)
                            nc.tensor.matmul(
                                psum_k[:, sl],
                                lhsT=wk_all[:, d, :],
                                rhs=xT_all[:, d, sl],
                                start=(d == 0), stop=False,
                            )
                        for nb in range(3):
                            sl = slice(nb * 512, (nb + 1) * 512)
                            nc.tensor.matmul(
                                psum_v[:, sl],
                                lhsT=wv_all[:, d, :],
                                rhs=xT_all[:, d, sl],
                                start=(d == 0), stop=False,
                            )
                        if d == 14:
                            # fill the DMA-starved tail of the chunk stream
                            q_proj(0, wq_tiles[0])
                    # rank-1 bias updates close each accumulation group
                    for nb in range(4):
                        sl = slice(nb * 512, (nb + 1) * 512)
                        nc.tensor.matmul(
                            psum_k[:, sl], lhsT=bkb[:, :],
                            rhs=ones_big[:, :], start=False, stop=True,
                        )
                    for nb in range(3):
                        sl = slice(nb * 512, (nb + 1) * 512)
                        nc.tensor.matmul(
                            psum_v[:, sl], lhsT=bvb[:, :],
                            rhs=ones_big[:, :], start=False, stop=True,
                        )
                    # drains split across DVE + ScalarE; kT block 0 first so
                    # its bank is free for v block 3's second pass below.
                    nc.vector.tensor_copy(kT[:, 0:512], psum_k[:, 0:512])
                    nc.scalar.activation(vT[:, 0:512], psum_v[:, 0:512], Copy)
                    nc.vector.tensor_copy(vT[:, 512:1024], psum_v[:, 512:1024])
                    nc.scalar.activation(vT[:, 1024:1536], psum_v[:, 1024:1536],
                                         Copy)
                    nc.vector.tensor_copy(kT[:, 512:1024], psum_k[:, 512:1024])
                    nc.scalar.activation(kT[:, 1024:1536], psum_k[:, 1024:1536],
                                         Copy)
                    nc.vector.tensor_copy(kT[:, 1536:2048], psum_k[:, 1536:2048])
                    # q1 covers the kT block-0 drain latency on the PE
                    q_proj(1, wq_tiles[1])
                    # v block 3 (keys 1536-2048): second pass through the
                    # freed k bank.  Pure PE work; the chunk data is SBUF
                    # resident, so this rides the DMA-bound region for free.
                    for d in range(ND):
                        nc.tensor.matmul(
                            psum_k[:, 0:512], lhsT=wv_all[:, d, :],
                            rhs=xT_all[:, d, 1536:2048],
                            start=(d == 0), stop=False,
                        )
                    nc.tensor.matmul(
                        psum_k[:, 0:512], lhsT=bvb[:, :],
                        rhs=ones_big[:, :], start=False, stop=True,
                    )
                    nc.scalar.activation(vT[:, 1536:2048], psum_k[:, 0:512],
                                         Copy)

            # ---- bootstrap: v transposes + bo broadcast --------------------
            with tc.tile_pool(name="ptr", bufs=1, space="PSUM") as ptrp:
                # copies split across VectorE/ScalarE: B1's first scores wait
                # on these PSUM banks, and a serial DVE backlog here was the
                # 3.5us stall (plus HAM re-throttle) at the B1 handoff.
                for t in range(NT):
                    ptr = ptrp.tile([128, 128], BF16, tag="tr", bufs=2)
                    nc.tensor.transpose(
                        ptr[:, :], vT[:, t * 128:(t + 1) * 128], ident[:, :]
                    )
                    if t % 2 == 0:
                        nc.vector.tensor_copy(v_nat[:, t, :], ptr[:, :])
                    else:
                        nc.scalar.activation(v_nat[:, t, :], ptr[:, :], Copy)
                # bias broadcast for phase C: bo_b = ones(128) x bo_row
                for nb in range(D // 512):
                    sl = slice(nb * 512, (nb + 1) * 512)
                    pbo = ptrp.tile([128, 512], F32, tag="bo", bufs=2)
                    nc.tensor.matmul(
                        pbo[:, :], lhsT=ones_fr[0:1, :], rhs=bo_row[0:1, sl],
                        start=True, stop=True,
                    )
                    if nb % 2 == 0:
                        nc.scalar.activation(bo_b[:, sl], pbo[:, :], Copy)
                    else:
                        nc.vector.tensor_copy(bo_b[:, sl], pbo[:, :])

            # ---- phase B1': fused attention + q-projection pipeline --------
            outT_list = [None] * HQ
            NS = HQ * NSH
            with (
                tc.tile_pool(name="psc", bufs=2, space="PSUM") as pscp,
                tc.tile_pool(name="po", bufs=2, space="PSUM") as pop,
                tc.tile_pool(name="aux", bufs=1, space="PSUM") as auxp,
            ):
                pT_t, P_t, Bt_t, Ct_t, root_t = {}, {}, {}, {}, {}
                po_t, rc_t = {}, {}

                def emit_scores(s):
                    h, tp = divmod(s, NSH)
                    psc = pscp.tile([128, 2 * SBLK], F32, tag="sc")
                    for half in range(2):
                        t = tp * 2 + half
                        nc.tensor.matmul(
                            psc[:, half * SBLK:(half + 1) * SBLK],
                            lhsT=kT[:, t * 128:(t + 1) * 128],
                            rhs=qT_all[:, h, :],
                            start=True, stop=True,
                        )
                    return psc

                def emit_exp(s, psc):
                    h, tp = divmod(s, NSH)
                    pT = ptpool.tile([128, 2 * SBLK], BF16, tag="pT")
                    nc.scalar.activation(pT[:, :], psc[:, :], Exp, scale=SCALE)
                    pT_t[s] = pT
                    # denominator: pairwise add + bf16 tree on DVE
                    P = dpool.tile([128, SBLK], BF16, tag="P", name=f"P{tp}")
                    nc.vector.tensor_tensor(
                        P[:, :], pT[:, 0:SBLK], pT[:, SBLK:2 * SBLK], add
                    )
                    P_t[tp] = P
                    if tp % 2 == 1:
                        Bt = dpool.tile([128, SBLK], BF16, tag="B",
                                        name=f"B{tp // 2}")
                        nc.vector.tensor_tensor(
                            Bt[:, :], P_t.pop(tp - 1)[:, :], P_t.pop(tp)[:, :],
                            add,
                        )
                        Bt_t[tp // 2] = Bt
                    if tp in (3, 7):
                        Ct = dpool.tile([128, SBLK], BF16, tag="C",
                                        name=f"C{tp // 4}")
                        nc.vector.tensor_tensor(
                            Ct[:, :], Bt_t.pop(tp // 2 - 1)[:, :],
                            Bt_t.pop(tp // 2)[:, :], add,
                        )
                        Ct_t[tp // 4] = Ct
                    if tp == 7:
                        root = dpool.tile([128, SBLK], BF16, tag="root",
                                          name=f"root{h}")
                        nc.vector.tensor_tensor(
                            root[:, :], Ct_t.pop(0)[:, :], Ct_t.pop(1)[:, :],
                            add,
                        )
                        root_t[h] = root

                def emit_pv(s):
                    h, tp = divmod(s, NSH)
                    if tp == 0:
                        po_t[h] = pop.tile([128, SBLK], F32, tag="po",
                                           name=f"po{h}")
                    pT = pT_t.pop(s)
                    for half in range(2):
                        t = tp * 2 + half
                        nc.tensor.matmul(
                            po_t[h][:, :],
                            lhsT=v_nat[:, t, :],
                            rhs=pT[:, half * SBLK:(half + 1) * SBLK],
                            start=(t == 0), stop=(t == NT - 1),
                        )

                def emit_tail1(h):
                    # partition-reduce the tree root + reciprocal.  Runs 3
                    # steps after the head's last scores so the DVE tree root
                    # is long done when the PE matmul wants it.
                    # lhsT is the full 128x128 ones tile: every output
                    # partition gets the same column sum, i.e. the reduce IS
                    # the broadcast, at the same N=512 matmul cost — and the
                    # 128-col LDWEIGHTS pipelines like every other weight
                    # load (a 1-col load breaks the weight-buffer rhythm).
                    pd = auxp.tile([128, SBLK], F32, tag="aux", name=f"pd{h}")
                    nc.tensor.matmul(
                        pd[:, :], lhsT=ones[:, :], rhs=root_t.pop(h)[:, :],
                        start=True, stop=True,
                    )
                    rb = rcpool.tile([128, SBLK], F32, tag="rb", name=f"rb{h}")
                    nc.vector.reciprocal_approx_fast(rb[:, :], pd[:, :])
                    rc_t[h] = rb

                def emit_tail2(h):
                    # normalize; 3 steps after tail1 so the PE never waits on
                    # the DVE->GpSimd round-trip.
                    outT = otpool.tile([128, SBLK], BF16, tag="ot",
                                       name=f"ot{h}")
                    nc.vector.tensor_tensor(
                        outT[:, :], po_t.pop(h)[:, :], rc_t.pop(h)[:, :], mult
                    )
                    outT_list[h] = outT

                def emit_fold(s):
                    # q-projection for head h+2, 2 matmuls per step
                    h, tp = divmod(s, NSH)
                    hf = h + 2
                    if hf >= HQ:
                        return
                    if tp == 0:
                        # queue DMA for the next folded head's weights
                        hn = hf + 1
                        if hn < HQ:
                            wq_tiles[hn] = wqpool.tile(
                                [128, ND, 128], BF16, tag="wq", name=f"wq{hn}"
                            )
                            nc.sync.dma_start(
                                out=wq_tiles[hn][:, :, :],
                                in_=Wq[:, hn * 128:(hn + 1) * 128].rearrange(
                                    "(n p) m -> p n m", p=128
                                ),
                            )
                        # stream this head's slice of Wo for phase C
                        if h < 8:
                            for hh in range(2):
                                hw = h * 2 + hh
                                for db in range(D // 512):
                                    dsl = slice(db * 512, (db + 1) * 512)
                                    wt = wopool.tile(
                                        [128, 512], BF16, tag="wo",
                                        name=f"wo{db}_{hw}"
                                    )
                                    nc.sync.dma_start(
                                        out=wt[:, :],
                                        in_=Wo[hw * 128:(hw + 1) * 128, dsl],
                                    )
                                    wo_tiles[db, hw] = wt
                        fold_pq[hf] = pqp.tile([128, SBLK], F32, tag="pq",
                                               name=f"pqf{hf}")
                    pq = fold_pq[hf]
                    for dd in range(2):
                        d = tp * 2 + dd
                        nc.tensor.matmul(
                            pq[:, :], lhsT=wq_tiles[hf][:, d, :],
                            rhs=xq[:, d, :], start=(d == 0), stop=False,
                        )
                    if tp == NSH - 1:
                        nc.tensor.matmul(
                            pq[:, :], lhsT=bqb[:, hf * 128:(hf + 1) * 128],
                            rhs=ones_big[:, :], start=False, stop=True,
                        )
                        nc.scalar.activation(qT_all[:, hf, :], pq[:, :], Copy)

                # Wo prefetch pool opens after phase-A SBUF is released
                wopool = tc.alloc_tile_pool(name="wo", bufs=64)
                wo_tiles = {}
                fold_pq = {}

                psc_t = {}
                for s in range(NS + 6):
                    if s < NS:
                        psc_t[s] = emit_scores(s)
                        emit_fold(s)
                    if s == 2:
                        # dependency-free filler: the pipeline has no p@v
                        # work yet and scores(2) waits on exp(0), so the PE
                        # would idle here (and HAM would re-throttle the
                        # clock).  Aux bank is unused until head 0's tail.
                        wa = auxp.tile([128, SBLK], F32, tag="aux",
                                       name="warmaux")
                        for w in range(6):
                            nc.tensor.matmul(
                                wa[:, :], lhsT=warm_lhs[:, :],
                                rhs=warm_rhs[:, :], start=True, stop=True,
                            )
                    if 0 <= s - 1 < NS:
                        emit_exp(s - 1, psc_t.pop(s - 1))
                    if 0 <= s - 2 < NS:
                        emit_pv(s - 2)
                    if 0 <= s - 3 < NS and (s - 3) % NSH == NSH - 1:
                        emit_tail1((s - 3) // NSH)
                    if 0 <= s - 6 < NS and (s - 6) % NSH == NSH - 1:
                        emit_tail2((s - 6) // NSH)

            # ---- phase C: output projection y = out @ Wo + bo --------------
            with tc.tile_pool(name="py", bufs=3, space="PSUM") as pyp:
                for db in range(D // 512):
                    dsl = slice(db * 512, (db + 1) * 512)
                    for st in range(NQ):
                        py = pyp.tile([128, 512], F32, tag="py")
                        for hh in range(HQ):
                            nc.tensor.matmul(
                                py[:, :],
                                lhsT=outT_list[hh][:, st * 128:(st + 1) * 128],
                                rhs=wo_tiles[db, hh][:, :],
                                start=(hh == 0), stop=(hh == HQ - 1),
                            )
                        y_sb = ypool.tile([128, 512], F32, tag="y")
                        nc.vector.tensor_tensor(
                            y_sb[:, :], py[:, :], bo_b[:, dsl], add
                        )
                        nc.sync.dma_start(
                            out=y[st * 128:(st + 1) * 128, dsl], in_=y_sb[:, :]
                        )

            wopool.release()
            pqp.release()

    nc.compile()
    return nc


def _get_nc():
    if "nc" not in _cache:
        _cache["nc"] = _build()
    return _cache["nc"]


def _prepare_in_maps(x, Wq, bq, Wk, bk, Wv, bv, Wo, bo):
    bf = ml_dtypes.bfloat16
    x = np.asarray(x, dtype=np.float32)
    bqT = (_round_fp32r(bq) / np.float32(128))[None, :]
    bkT = (_round_fp32r(bk) / np.float32(128))[None, :]
    bvT = (_round_fp32r(bv) / np.float32(128))[None, :]
    bo = _round_fp32r(bo)[None, :]
    Wq_b = np.asarray(Wq, np.float32).astype(bf)
    Wk_b = np.asarray(Wk, np.float32).astype(bf)
    Wv_b = np.asarray(Wv, np.float32).astype(bf)
    Wo_b = np.asarray(Wo, np.float32).astype(bf)
    ones = np.ones((128, 128), bf)
    onesf = np.ones((1, 128), np.float32)

    xT = [np.ascontiguousarray(x[g].T).astype(bf) for g in range(B)]
    in_maps = []
    for c in range(N_CORES):
        g, blk = divmod(c, 4)
        s0 = blk * SBLK
        in_maps.append({
            "xT": xT[g],
            "xTq": np.ascontiguousarray(xT[g][:, s0:s0 + SBLK]),
            "Wq": Wq_b, "Wk": Wk_b, "Wv": Wv_b, "Wo": Wo_b,
            "bqT": bqT, "bkT": bkT, "bvT": bvT, "bo": bo,
            "ones": ones, "onesf": onesf,
        })
    return in_maps


def _assemble(results):
    out = np.empty((B, S, D), dtype=np.float32)
    for c in range(N_CORES):
        g, blk = divmod(c, 4)
        out[g, blk * SBLK:(blk + 1) * SBLK, :] = results[c]["y"]
    return out


def kernel(x, Wq, bq, Wk, bk, Wv, bv, Wo, bo):
    from concourse.bass_utils import run_bass_kernel_spmd

    in_maps = _prepare_in_maps(x, Wq, bq, Wk, bk, Wv, bv, Wo, bo)
    nc = _get_nc()
    res = run_bass_kernel_spmd(nc, in_maps, core_ids=list(range(N_CORES)))
    return _assemble(res.results)


# revision 43
# speedup vs baseline: 1.0124x; 1.0022x over previous
"""Multi-head attention block (16 query heads, shared single K/V head) on
8 Trainium2 NeuronCores.

Reference computation (B=2, S=2048, D=2048, HQ=16, DH=128, fp32):
    q = (x @ Wq + bq)  -> [B, S, 16, 128]
    k = x @ Wk + bk    -> [B, S, 128]   (single shared K/V head)
    v = x @ Wv + bv    -> [B, S, 128]
    attn = softmax(q k^T / sqrt(128))
    out = (attn @ v) reshaped -> [B, S, D];  y = out @ Wo + bo

Sharding: batch x sequence-block data parallel. Core c handles batch c//4,
query rows (c%4)*512 .. +512, for ALL 16 heads. No inter-core collectives;
every core emits a disjoint slab of the final output.

All matmuls run in bfloat16 (fp32 accumulation in PSUM). Schedule (v2,
fused pipeline):

  warm : 16 dummy matmuls on memset tiles (no DMA deps) keep the PE busy
         while the first DMAs land, so HAM un-throttles (1.2->2.4 GHz)
         before real work.
  A    : k/v projections over the full sequence, d-chunk streamed from HBM
         (DMA-paced; xT chunks keep queue priority, xq/wq tiles slot in
         late). All biases are folded into the matmuls as rank-1
         [1,x] @ [1,N] updates so the PSUM->SBUF drains are pure copies,
         split across ScalarE and VectorE. The drain whose bank the next
         phase needs goes first.
  boot : q-projection for heads 0-1 + PE transposes of v into [key,dh]
         layout + bias broadcast for the output projection.
  B1'  : fused per-head attention x q-projection pipeline, 8 steps per
         head. Each step: 2 scores MMs (PE) -> exp (ScalarE) -> 2 p@v MMs
         (PE, 2 steps later) + 2 q-projection MMs for head h+2 (PE) +
         bf16 tree accumulation of the softmax denominator (VectorE).
         Per head: one [128,1]-ones matmul reduces the tree root over
         partitions (+3-step skew), reciprocal (VectorE) and a partition
         broadcast on the otherwise-idle GpSimd engine, then normalize
         (+3 more steps of skew so the PE never waits on the round-trip).
         ScalarE also drains each folded head's q tile (pure copy).
  C    : output projection y = out @ Wo + bo with Wo prefetched to SBUF
         during B1'. First chain overlaps B1's tail (its PSUM banks are
         the score banks the last exp freed).

Measured: 303-306us on HW (baseline schedule: 360us), rel err 5.36e-3.
Schedules tried and rejected: pv-before-scores emission (+5us: head-of-
line blocking on the 2-step-old exp), front-loading head 2's fold into
the first steps (neutral), early xq/wq DMA placement (starves the xT
stream mid-phase-A and re-throttles HAM).

Final measured: 295.3-298.4us across 5 clean runs (baseline: 360.3us),
rel err 5.36e-3.  NOTE: many back-to-back runs downclock the whole chip
1.2x (MM gap 215->258ns, exp 1114->1336ns, HAM still K=8/8); ~5min idle
recovers.  LANDED: full-K bias matmuls (biases arrive /128 in f32r --
exact -- and are replicated to [128,x] bf16 via PE broadcast matmuls
through the warm-up PSUM bank; the rank-1 1-partition lhsT version cost
two stalls per folded head).  B1' now 161.7us with 26 stalls >=300ns
(was 164.8us / 39); totals 295.2/297.4/298.2us (best-of-session 295.2
after a cool-down; the higher two were back-to-back thermally-inflated
runs).  WARNING: do
NOT replicate via gpsimd.partition_broadcast on BF16 [1,2048] -- it
corrupts memory (rel err 0.269 with zero biases); the proven gpsimd
broadcasts were FP32 [1,512].  Remaining per-head stall: one ~432ns gap at the
tail1 iteration (tp2) -- the only 7-matmul iteration; the extra pd MM
rotates the LDWEIGHTS weight-buffer-WAR phase so one load lands outside
its hiding window.  Shifting the skew relocates, not removes, it
(~2-3us, floor-adjacent).  Also rejected: AllGather k/v dedup (72.6us cc-active per 1MB in
this harness); dma_start_transpose for v (neutral); warm-matmul fillers
in DMA-bound phase A (3 attempts, each +1..2us); fold-before-scores
order (neutral); psc bufs=3 (no PSUM left).
"""

import numpy as np
import ml_dtypes

B, S, D = 2, 2048, 2048
HQ, DH = 16, 128
SBLK = S // 4          # 512 query rows per core
N_CORES = 8
SCALE = 1.0 / float(np.sqrt(DH))

ND = D // 128          # 16 contraction chunks
NT = S // 128          # 16 key tiles
NQ = SBLK // 128       # 4 query row-tiles per core
NSH = NT // 2          # 8 pipeline steps per head

_cache = {}


def _round_fp32r(a):
    """Round fp32 to fp32r (1s+8e+11m) with round-to-nearest-even-ish."""
    b = np.ascontiguousarray(a, dtype=np.float32).view(np.uint32)
    bias = np.uint32(0x7FF) + ((b >> np.uint32(12)) & np.uint32(1))
    return ((b + bias) & np.uint32(0xFFFFF000)).view(np.float32)


def _build():
    from concourse import bacc, mybir, tile
    from concourse.masks import make_identity

    F32 = mybir.dt.float32
    F32R = mybir.dt.float32r
    BF16 = mybir.dt.bfloat16
    Exp = mybir.ActivationFunctionType.Exp
    Copy = mybir.ActivationFunctionType.Copy
    mult = mybir.AluOpType.mult
    add = mybir.AluOpType.add

    nc = bacc.Bacc("TRN2", target_bir_lowering=False, debug=False,
                   num_devices=N_CORES)

    xT = nc.dram_tensor("xT", [D, S], BF16, kind="ExternalInput").ap()
    xTq = nc.dram_tensor("xTq", [D, SBLK], BF16, kind="ExternalInput").ap()
    Wq = nc.dram_tensor("Wq", [D, D], BF16, kind="ExternalInput").ap()
    Wk = nc.dram_tensor("Wk", [D, DH], BF16, kind="ExternalInput").ap()
    Wv = nc.dram_tensor("Wv", [D, DH], BF16, kind="ExternalInput").ap()
    Wo = nc.dram_tensor("Wo", [D, D], BF16, kind="ExternalInput").ap()
    bqT_d = nc.dram_tensor("bqT", [1, D], F32R, kind="ExternalInput").ap()
    bkT_d = nc.dram_tensor("bkT", [1, DH], F32R, kind="ExternalInput").ap()
    bvT_d = nc.dram_tensor("bvT", [1, DH], F32R, kind="ExternalInput").ap()
    bo_d = nc.dram_tensor("bo", [1, D], F32R, kind="ExternalInput").ap()
    ones_d = nc.dram_tensor("ones", [128, 128], BF16, kind="ExternalInput").ap()
    ones_fd = nc.dram_tensor("onesf", [1, 128], F32R, kind="ExternalInput").ap()
    y = nc.dram_tensor("y", [SBLK, D], F32, kind="ExternalOutput").ap()

    with tile.TileContext(nc) as tc, nc.allow_low_precision(
        reason="bf16 matmul pipeline; verified against fp32 reference"
    ):
        with (
            tc.tile_pool(name="const", bufs=1) as cpool,
            tc.tile_pool(name="live", bufs=1) as lpool,      # kT, vT, v_nat, xq, qT
            tc.tile_pool(name="ot", bufs=HQ) as otpool,      # 16 head outputs
            tc.tile_pool(name="wq", bufs=4) as wqpool,       # Wq stream
            tc.tile_pool(name="pt", bufs=4) as ptpool,       # exp outputs
            tc.tile_pool(name="den", bufs=2) as dpool,       # denominator tree
            tc.tile_pool(name="rc", bufs=2) as rcpool,       # recip + broadcast
            tc.tile_pool(name="yp", bufs=3) as ypool,
        ):
            # ---- constants -------------------------------------------------
            ones = cpool.tile([128, 128], BF16)
            nc.sync.dma_start(out=ones[:, :], in_=ones_d[:, :])
            ones_col = ones[:, 0:1]
            ones_fr = cpool.tile([1, 128], F32R)
            nc.sync.dma_start(out=ones_fr[:, :], in_=ones_fd[:, :])
            bkT = cpool.tile([1, DH], F32R)
            nc.sync.dma_start(out=bkT[:, :], in_=bkT_d[:, :])
            bvT = cpool.tile([1, DH], F32R)
            nc.sync.dma_start(out=bvT[:, :], in_=bvT_d[:, :])
            bqT = cpool.tile([1, D], F32R)
            nc.sync.dma_start(out=bqT[:, :], in_=bqT_d[:, :])
            bo_row = cpool.tile([1, D], F32R)
            nc.sync.dma_start(out=bo_row[:, :], in_=bo_d[:, :])

            ones_row = cpool.tile([1, SBLK], BF16)
            nc.vector.memset(ones_row[:, :], 1.0)
            ones_big = cpool.tile([128, SBLK], BF16)
            nc.vector.memset(ones_big[:, :], 1.0)
            # biases replicated across partitions (values arrive /128, exact)
            # so bias matmuls use a full 128x128 stationary operand -- the
            # rank-1 1-partition lhsT broke PE weight-buffer pipelining
            # (~310+545ns of stalls per folded head).  Built below with PE
            # broadcast matmuls (the proven bo_b pattern; GpSimd
            # partition_broadcast corrupts bf16/[1,2048] operands).
            bqb = cpool.tile([128, D], BF16)
            bkb = cpool.tile([128, DH], BF16)
            bvb = cpool.tile([128, DH], BF16)
            warm_rhs = cpool.tile([128, SBLK], BF16)
            nc.vector.memset(warm_rhs[:, :], 0.0)
            warm_lhs = cpool.tile([128, 128], BF16)
            nc.vector.memset(warm_lhs[:, :], 0.0)
            ident = cpool.tile([128, 128], BF16)
            make_identity(nc, ident[:, :])
            bo_b = cpool.tile([128, D], F32)

            warm = cpool.tile([1, 1], BF16)
            nc.scalar.activation(warm[:, :], ones[0:1, 0:1], Exp, scale=1.0)

            kT = lpool.tile([128, S], BF16)
            vT = lpool.tile([128, S], BF16)
            v_nat = lpool.tile([128, NT, DH], BF16)
            xq = lpool.tile([128, ND, SBLK], BF16)
            qT_all = lpool.tile([128, HQ, SBLK], BF16)

            # q-projection PSUM bank allocated BEFORE phase A's pool so the
            # bootstrap q-projections never wait on the pool-wide barrier
            # against phase A's eight PSUM drains.
            pqp = tc.alloc_tile_pool(name="pq", bufs=1, space="PSUM")

            def q_proj(h, wq_t):
                pq = pqp.tile([128, SBLK], F32, tag="pq", name=f"pq{h}")
                for d in range(ND):
                    nc.tensor.matmul(
                        pq[:, :], lhsT=wq_t[:, d, :], rhs=xq[:, d, :],
                        start=(d == 0), stop=False,
                    )
                nc.tensor.matmul(
                    pq[:, :], lhsT=bqb[:, h * 128:(h + 1) * 128],
                    rhs=ones_big[:, :], start=False, stop=True,
                )
                nc.scalar.activation(qT_all[:, h, :], pq[:, :], Copy)

            # ---- phase A: k/v projections over the full sequence -----------
            with tc.tile_pool(name="pha", bufs=1) as apool:
                wk_all = apool.tile([128, ND, DH], BF16)
                nc.sync.dma_start(
                    out=wk_all[:, :, :],
                    in_=Wk.rearrange("(n p) d -> p n d", p=128),
                )
                wv_all = apool.tile([128, ND, DH], BF16)
                nc.sync.dma_start(
                    out=wv_all[:, :, :],
                    in_=Wv.rearrange("(n p) d -> p n d", p=128),
                )
                # xT chunks stream in consumption order; xq and the first
                # q-weight tiles are interleaved late enough not to delay the
                # matmul stream (PE has buffered chunks by then) but early
                # enough to be resident when the bootstrap needs them.
                wq_tiles = {}

                def wq_dma(h):
                    wq_tiles[h] = wqpool.tile([128, ND, 128], BF16, tag="wq",
                                              name=f"wq{h}")
                    nc.sync.dma_start(
                        out=wq_tiles[h][:, :, :],
                        in_=Wq[:, h * 128:(h + 1) * 128].rearrange(
                            "(n p) m -> p n m", p=128
                        ),
                    )

                # xT chunks keep DMA-queue priority (they pace phase A); the
                # xq quarters and first wq tiles slot in late, where the PE
                # has buffered chunks, and land just before the bootstrap
                # needs them.
                xT_all = apool.tile([128, ND, S], BF16)
                xq_quarter = {9: 0, 10: 1, 12: 2, 13: 3}
                for d in range(ND):
                    nc.sync.dma_start(
                        out=xT_all[:, d, :], in_=xT[d * 128:(d + 1) * 128, :]
                    )
                    if d in xq_quarter:
                        q = xq_quarter[d]
                        nc.sync.dma_start(
                            out=xq[:, 4 * q:4 * (q + 1), :],
                            in_=xTq[512 * q:512 * (q + 1), :].rearrange(
                                "(n p) s -> p n s", p=128
                            ),
                        )
                    if d == 11:
                        wq_dma(0)
                    elif d == 15:
                        wq_dma(1)
                        wq_dma(2)

                with tc.tile_pool(name="pacc", bufs=1, space="PSUM") as pacc:
                    psum_k = pacc.tile([128, S], F32, tag="pk")
                    psum_v = pacc.tile([128, 1536], F32, tag="pv")

                    # PE warm-up in the (pre-allocated) q bank: keep the
                    # array busy while DMAs land so the HAM clock gate opens
                    # before the real stream begins.  No DMA dependencies.
                    warm_t = pqp.tile([128, SBLK], F32, tag="pq",
                                      name="warmpq")
                    for w in range(16):
                        nc.tensor.matmul(
                            warm_t[:, :],
                            lhsT=warm_lhs[:, :], rhs=warm_rhs[:, :],
                            start=True, stop=True,
                        )
                    # replicate biases across partitions (DMA-bound window,
                    # the PE/DVE round-trips here are free)
                    for j in range(D // 512):
                        nc.tensor.matmul(
                            warm_t[:, :], lhsT=ones_fr[0:1, :],
                            rhs=bqT[0:1, j * 512:(j + 1) * 512],
                            start=True, stop=True,
                        )
                        nc.vector.tensor_copy(
                            bqb[:, j * 512:(j + 1) * 512], warm_t[:, :]
                        )
                    nc.tensor.matmul(
                        warm_t[:, 0:DH], lhsT=ones_fr[0:1, :],
                        rhs=bkT[0:1, :], start=True, stop=True,
                    )
                    nc.vector.tensor_copy(bkb[:, :], warm_t[:, 0:DH])
                    nc.tensor.matmul(
                        warm_t[:, 0:DH], lhsT=ones_fr[0:1, :],
                        rhs=bvT[0:1, :], start=True, stop=True,
                    )
                    nc.vector.tensor_copy(bvb[:, :], warm_t[:, 0:DH])

                    for d in range(ND):
                        for nb in range(4):
                            sl = slice(nb * 512, (nb + 1) * 512)
                            nc.tensor.matmul(
                                psum_k[:, sl],
                                lhsT=wk_all[:, d, :],
                                rhs=xT_all[:, d, sl],
                                start=(d == 0), stop=False,
                            )
                        for nb in range(3):
                            sl = slice(nb * 512, (nb + 1) * 512)
                            nc.tensor.matmul(
                                psum_v[:, sl],
                                lhsT=wv_all[:, d, :],
                                rhs=xT_all[:, d, sl],
                                start=(d == 0), stop=False,
                            )
                        if d == 14:
                            # fill the DMA-starved tail of the chunk stream
                            q_proj(0, wq_tiles[0])
                    # rank-1 bias updates close each accumulation group
                    for nb in range(4):
                        sl = slice(nb * 512, (nb + 1) * 512)
                        nc.tensor.matmul(
                            psum_k[:, sl], lhsT=bkb[:, :],
                            rhs=ones_big[:, :], start=False, stop=True,
                        )
                    for nb in range(3):
                        sl = slice(nb * 512, (nb + 1) * 512)
                        nc.tensor.matmul(
                            psum_v[:, sl], lhsT=bvb[:, :],
                            rhs=ones_big[:, :], start=False, stop=True,
                        )
                    # drains split across DVE + ScalarE; kT block 0 first so
                    # its bank is free for v block 3's second pass below.
                    nc.vector.tensor_copy(kT[:, 0:512], psum_k[:, 0:512])
                    nc.scalar.activation(vT[:, 0:512], psum_v[:, 0:512], Copy)
                    nc.vector.tensor_copy(vT[:, 512:1024], psum_v[:, 512:1024])
                    nc.scalar.activation(vT[:, 1024:1536], psum_v[:, 1024:1536],
                                         Copy)
                    nc.vector.tensor_copy(kT[:, 512:1024], psum_k[:, 512:1024])
                    nc.scalar.activation(kT[:, 1024:1536], psum_k[:, 1024:1536],
                                         Copy)
                    nc.vector.tensor_copy(kT[:, 1536:2048], psum_k[:, 1536:2048])
                    # q1 covers the kT block-0 drain latency on the PE
                    q_proj(1, wq_tiles[1])
                    # v block 3 (keys 1536-2048): second pass through the
                    # freed k bank.  Pure PE work; the chunk data is SBUF
                    # resident, so this rides the DMA-bound region for free.
                    for d in range(ND):
                        nc.tensor.matmul(
                            psum_k[:, 0:512], lhsT=wv_all[:, d, :],
                            rhs=xT_all[:, d, 1536:2048],
                            start=(d == 0), stop=False,
                        )
                    nc.tensor.matmul(
                        psum_k[:, 0:512], lhsT=bvb[:, :],
                        rhs=ones_big[:, :], start=False, stop=True,
                    )
                    nc.scalar.activation(vT[:, 1536:2048], psum_k[:, 0:512],
                                         Copy)

            # ---- bootstrap: v transposes + bo broadcast --------------------
            with tc.tile_pool(name="ptr", bufs=1, space="PSUM") as ptrp:
                # copies split across VectorE/ScalarE: B1's first scores wait
                # on these PSUM banks, and a serial DVE backlog here was the
                # 3.5us stall (plus HAM re-throttle) at the B1 handoff.
                for t in range(NT):
                    ptr = ptrp.tile([128, 128], BF16, tag="tr", bufs=2)
                    nc.tensor.transpose(
                        ptr[:, :], vT[:, t * 128:(t + 1) * 128], ident[:, :]
                    )
                    if t % 2 == 0:
                        nc.vector.tensor_copy(v_nat[:, t, :], ptr[:, :])
                    else:
                        nc.scalar.activation(v_nat[:, t, :], ptr[:, :], Copy)
                # bias broadcast for phase C: bo_b = ones(128) x bo_row
                for nb in range(D // 512):
                    sl = slice(nb * 512, (nb + 1) * 512)
                    pbo = ptrp.tile([128, 512], F32, tag="bo", bufs=2)
                    nc.tensor.matmul(
                        pbo[:, :], lhsT=ones_fr[0:1, :], rhs=bo_row[0:1, sl],
                        start=True, stop=True,
                    )
                    if nb % 2 == 0:
                        nc.scalar.activation(bo_b[:, sl], pbo[:, :], Copy)
                    else:
                        nc.vector.tensor_copy(bo_b[:, sl], pbo[:, :])

            # ---- phase B1': fused attention + q-projection pipeline --------
            outT_list = [None] * HQ
            NS = HQ * NSH
            with (
                tc.tile_pool(name="psc", bufs=2, space="PSUM") as pscp,
                tc.tile_pool(name="po", bufs=2, space="PSUM") as pop,
                tc.tile_pool(name="aux", bufs=1, space="PSUM") as auxp,
            ):
                pT_t, P_t, Bt_t, Ct_t, root_t = {}, {}, {}, {}, {}
                po_t, rc_t = {}, {}

                def emit_scores(s):
                    h, tp = divmod(s, NSH)
                    psc = pscp.tile([128, 2 * SBLK], F32, tag="sc")
                    for half in range(2):
                        t = tp * 2 + half
                        nc.tensor.matmul(
                            psc[:, half * SBLK:(half + 1) * SBLK],
                            lhsT=kT[:, t * 128:(t + 1) * 128],
                            rhs=qT_all[:, h, :],
                            start=True, stop=True,
                        )
                    return psc

                def emit_exp(s, psc):
                    h, tp = divmod(s, NSH)
                    pT = ptpool.tile([128, 2 * SBLK], BF16, tag="pT")
                    nc.scalar.activation(pT[:, :], psc[:, :], Exp, scale=SCALE)
                    pT_t[s] = pT
                    # denominator: pairwise add + bf16 tree on DVE
                    P = dpool.tile([128, SBLK], BF16, tag="P", name=f"P{tp}")
                    nc.vector.tensor_tensor(
                        P[:, :], pT[:, 0:SBLK], pT[:, SBLK:2 * SBLK], add
                    )
                    P_t[tp] = P
                    if tp % 2 == 1:
                        Bt = dpool.tile([128, SBLK], BF16, tag="B",
                                        name=f"B{tp // 2}")
                        nc.vector.tensor_tensor(
                            Bt[:, :], P_t.pop(tp - 1)[:, :], P_t.pop(tp)[:, :],
                            add,
                        )
                        Bt_t[tp // 2] = Bt
                    if tp in (3, 7):
                        Ct = dpool.tile([128, SBLK], BF16, tag="C",
                                        name=f"C{tp // 4}")
                        nc.vector.tensor_tensor(
                            Ct[:, :], Bt_t.pop(tp // 2 - 1)[:, :],
                            Bt_t.pop(tp // 2)[:, :], add,
                        )
                        Ct_t[tp // 4] = Ct
                    if tp == 7:
                        root = dpool.tile([128, SBLK], BF16, tag="root",
                                          name=f"root{h}")
                        nc.vector.tensor_tensor(
                            root[:, :], Ct_t.pop(0)[:, :], Ct_t.pop(1)[:, :],
                            add,
                        )
                        root_t[h] = root

                def emit_pv(s):
                    h, tp = divmod(s, NSH)
                    if tp == 0:
                        po_t[h] = pop.tile([128, SBLK], F32, tag="po",
                                           name=f"po{h}")
                    pT = pT_t.pop(s)
                    for half in range(2):
                        t = tp * 2 + half
                        nc.tensor.matmul(
                            po_t[h][:, :],
                            lhsT=v_nat[:, t, :],
                            rhs=pT[:, half * SBLK:(half + 1) * SBLK],
                            start=(t == 0), stop=(t == NT - 1),
                        )

                def emit_tail1(h):
                    # partition-reduce the tree root + reciprocal.  Runs 3
                    # steps after the head's last scores so the DVE tree root
                    # is long done when the PE matmul wants it.
                    # lhsT is the full 128x128 ones tile: every output
                    # partition gets the same column sum, i.e. the reduce IS
                    # the broadcast, at the same N=512 matmul cost — and the
                    # 128-col LDWEIGHTS pipelines like every other weight
                    # load (a 1-col load breaks the weight-buffer rhythm).
                    pd = auxp.tile([128, SBLK], F32, tag="aux", name=f"pd{h}")
                    nc.tensor.matmul(
                        pd[:, :], lhsT=ones[:, :], rhs=root_t.pop(h)[:, :],
                        start=True, stop=True,
                    )
                    rb = rcpool.tile([128, SBLK], F32, tag="rb", name=f"rb{h}")
                    nc.vector.reciprocal_approx_fast(rb[:, :], pd[:, :])
                    rc_t[h] = rb

                def emit_tail2(h):
                    # normalize; 3 steps after tail1 so the PE never waits on
                    # the DVE->GpSimd round-trip.
                    outT = otpool.tile([128, SBLK], BF16, tag="ot",
                                       name=f"ot{h}")
                    nc.vector.tensor_tensor(
                        outT[:, :], po_t.pop(h)[:, :], rc_t.pop(h)[:, :], mult
                    )
                    outT_list[h] = outT

                def emit_fold(s):
                    # q-projection for head h+2, 2 matmuls per step
                    h, tp = divmod(s, NSH)
                    hf = h + 2
                    if hf >= HQ:
                        return
                    if tp == 0:
                        # queue DMA for the next folded head's weights
                        hn = hf + 1
                        if hn < HQ:
                            wq_tiles[hn] = wqpool.tile(
                                [128, ND, 128], BF16, tag="wq", name=f"wq{hn}"
                            )
                            nc.sync.dma_start(
                                out=wq_tiles[hn][:, :, :],
                                in_=Wq[:, hn * 128:(hn + 1) * 128].rearrange(
                                    "(n p) m -> p n m", p=128
                                ),
                            )
                        # stream this head's slice of Wo for phase C
                        if h < 8:
                            for hh in range(2):
                                hw = h * 2 + hh
                                for db in range(D // 512):
                                    dsl = slice(db * 512, (db + 1) * 512)
                                    wt = wopool.tile(
                                        [128, 512], BF16, tag="wo",
                                        name=f"wo{db}_{hw}"
                                    )
                                    nc.sync.dma_start(
                                        out=wt[:, :],
                                        in_=Wo[hw * 128:(hw + 1) * 128, dsl],
                                    )
                                    wo_tiles[db, hw] = wt
                        fold_pq[hf] = pqp.tile([128, SBLK], F32, tag="pq",
                                               name=f"pqf{hf}")
                    pq = fold_pq[hf]
                    # tp2 carries the previous head's pd reduce matmul; give
                    # it only 1 fold MM (3 in tp1) so no iteration exceeds 6
                    # matmuls -- the 7-MM tail1 iteration rotated the
                    # LDWEIGHTS weight-buffer-WAR phase into a ~432ns stall.
                    counts = (2, 3, 1, 2, 2, 2, 2, 2)
                    d0 = sum(counts[:tp])
                    for d in range(d0, d0 + counts[tp]):
                        nc.tensor.matmul(
                            pq[:, :], lhsT=wq_tiles[hf][:, d, :],
                            rhs=xq[:, d, :], start=(d == 0), stop=False,
                        )
                    if tp == NSH - 1:
                        nc.tensor.matmul(
                            pq[:, :], lhsT=bqb[:, hf * 128:(hf + 1) * 128],
                            rhs=ones_big[:, :], start=False, stop=True,
                        )
                        nc.scalar.activation(qT_all[:, hf, :], pq[:, :], Copy)

                # Wo prefetch pool opens after phase-A SBUF is released
                wopool = tc.alloc_tile_pool(name="wo", bufs=64)
                wo_tiles = {}
                fold_pq = {}

                psc_t = {}
                for s in range(NS + 6):
                    if s < NS:
                        psc_t[s] = emit_scores(s)
                        emit_fold(s)
                    if s == 2:
                        # dependency-free filler: the pipeline has no p@v
                        # work yet and scores(2) waits on exp(0), so the PE
                        # would idle here (and HAM would re-throttle the
                        # clock).  Aux bank is unused until head 0's tail.
                        wa = auxp.tile([128, SBLK], F32, tag="aux",
                                       name="warmaux")
                        for w in range(6):
                            nc.tensor.matmul(
                                wa[:, :], lhsT=warm_lhs[:, :],
                                rhs=warm_rhs[:, :], start=True, stop=True,
                            )
                    if 0 <= s - 1 < NS:
                        emit_exp(s - 1, psc_t.pop(s - 1))
                    if 0 <= s - 2 < NS:
                        emit_pv(s - 2)
                    if 0 <= s - 3 < NS and (s - 3) % NSH == NSH - 1:
                        emit_tail1((s - 3) // NSH)
                    if 0 <= s - 6 < NS and (s - 6) % NSH == NSH - 1:
                        emit_tail2((s - 6) // NSH)

            # ---- phase C: output projection y = out @ Wo + bo --------------
            with tc.tile_pool(name="py", bufs=3, space="PSUM") as pyp:
                for db in range(D // 512):
                    dsl = slice(db * 512, (db + 1) * 512)
                    for st in range(NQ):
                        py = pyp.tile([128, 512], F32, tag="py")
                        for hh in range(HQ):
                            nc.tensor.matmul(
                                py[:, :],
                                lhsT=outT_list[hh][:, st * 128:(st + 1) * 128],
                                rhs=wo_tiles[db, hh][:, :],
                                start=(hh == 0), stop=(hh == HQ - 1),
                            )
                        y_sb = ypool.tile([128, 512], F32, tag="y")
                        nc.vector.tensor_tensor(
                            y_sb[:, :], py[:, :], bo_b[:, dsl], add
                        )
                        nc.sync.dma_start(
                            out=y[st * 128:(st + 1) * 128, dsl], in_=y_sb[:, :]
                        )

            wopool.release()
            pqp.release()

    nc.compile()
    return nc


def _get_nc():
    if "nc" not in _cache:
        _cache["nc"] = _build()
    return _cache["nc"]


def _prepare_in_maps(x, Wq, bq, Wk, bk, Wv, bv, Wo, bo):
    bf = ml_dtypes.bfloat16
    x = np.asarray(x, dtype=np.float32)
    bqT = (_round_fp32r(bq) / np.float32(128))[None, :]
    bkT = (_round_fp32r(bk) / np.float32(128))[None, :]
    bvT = (_round_fp32r(bv) / np.float32(128))[None, :]
    bo = _round_fp32r(bo)[None, :]
    Wq_b = np.asarray(Wq, np.float32).astype(bf)
    Wk_b = np.asarray(Wk, np.float32).astype(bf)
    Wv_b = np.asarray(Wv, np.float32).astype(bf)
    Wo_b = np.asarray(Wo, np.float32).astype(bf)
    ones = np.ones((128, 128), bf)
    onesf = np.ones((1, 128), np.float32)

    xT = [np.ascontiguousarray(x[g].T).astype(bf) for g in range(B)]
    in_maps = []
    for c in range(N_CORES):
        g, blk = divmod(c, 4)
        s0 = blk * SBLK
        in_maps.append({
            "xT": xT[g],
            "xTq": np.ascontiguousarray(xT[g][:, s0:s0 + SBLK]),
            "Wq": Wq_b, "Wk": Wk_b, "Wv": Wv_b, "Wo": Wo_b,
            "bqT": bqT, "bkT": bkT, "bvT": bvT, "bo": bo,
            "ones": ones, "onesf": onesf,
        })
    return in_maps


def _assemble(results):
    out = np.empty((B, S, D), dtype=np.float32)
    for c in range(N_CORES):
        g, blk = divmod(c, 4)
        out[g, blk * SBLK:(blk + 1) * SBLK, :] = results[c]["y"]
    return out


def kernel(x, Wq, bq, Wk, bk, Wv, bv, Wo, bo):
    from concourse.bass_utils import run_bass_kernel_spmd

    in_maps = _prepare_in_maps(x, Wq, bq, Wk, bk, Wv, bv, Wo, bo)
    nc = _get_nc()
    res = run_bass_kernel_spmd(nc, in_maps, core_ids=list(range(N_CORES)))
    return _assemble(res.results)


# revision 45
# speedup vs baseline: 1.0205x; 1.0080x over previous
"""Multi-head attention block (16 query heads, shared single K/V head) on
8 Trainium2 NeuronCores.

Reference computation (B=2, S=2048, D=2048, HQ=16, DH=128, fp32):
    q = (x @ Wq + bq)  -> [B, S, 16, 128]
    k = x @ Wk + bk    -> [B, S, 128]   (single shared K/V head)
    v = x @ Wv + bv    -> [B, S, 128]
    attn = softmax(q k^T / sqrt(128))
    out = (attn @ v) reshaped -> [B, S, D];  y = out @ Wo + bo

Sharding: batch x sequence-block data parallel. Core c handles batch c//4,
query rows (c%4)*512 .. +512, for ALL 16 heads. No inter-core collectives;
every core emits a disjoint slab of the final output.

All matmuls run in bfloat16 (fp32 accumulation in PSUM). Schedule (v2,
fused pipeline):

  warm : 16 dummy matmuls on memset tiles (no DMA deps) keep the PE busy
         while the first DMAs land, so HAM un-throttles (1.2->2.4 GHz)
         before real work.
  A    : k/v projections over the full sequence, d-chunk streamed from HBM
         (DMA-paced; xT chunks keep queue priority, xq/wq tiles slot in
         late). All biases are folded into the matmuls as rank-1
         [1,x] @ [1,N] updates so the PSUM->SBUF drains are pure copies,
         split across ScalarE and VectorE. The drain whose bank the next
         phase needs goes first.
  boot : q-projection for heads 0-1 + PE transposes of v into [key,dh]
         layout + bias broadcast for the output projection.
  B1'  : fused per-head attention x q-projection pipeline, 8 steps per
         head. Each step: 2 scores MMs (PE) -> exp (ScalarE) -> 2 p@v MMs
         (PE, 2 steps later) + 2 q-projection MMs for head h+2 (PE) +
         bf16 tree accumulation of the softmax denominator (VectorE).
         Per head: one [128,1]-ones matmul reduces the tree root over
         partitions (+3-step skew), reciprocal (VectorE) and a partition
         broadcast on the otherwise-idle GpSimd engine, then normalize
         (+3 more steps of skew so the PE never waits on the round-trip).
         ScalarE also drains each folded head's q tile (pure copy).
  C    : output projection y = out @ Wo + bo with Wo prefetched to SBUF
         during B1'. First chain overlaps B1's tail (its PSUM banks are
         the score banks the last exp freed).

Measured: 303-306us on HW (baseline schedule: 360us), rel err 5.36e-3.
Schedules tried and rejected: pv-before-scores emission (+5us: head-of-
line blocking on the 2-step-old exp), front-loading head 2's fold into
the first steps (neutral), early xq/wq DMA placement (starves the xT
stream mid-phase-A and re-throttles HAM).

Final measured: 295.3-298.4us across 5 clean runs (baseline: 360.3us),
rel err 5.36e-3.  NOTE: many back-to-back runs downclock the whole chip
1.2x (MM gap 215->258ns, exp 1114->1336ns, HAM still K=8/8); ~5min idle
recovers.  LANDED: full-K bias matmuls (biases arrive /128 in f32r --
exact -- and are replicated to [128,x] bf16 via PE broadcast matmuls
through the warm-up PSUM bank; the rank-1 1-partition lhsT version cost
two stalls per folded head).  B1' now 161.7us with 26 stalls >=300ns
(was 164.8us / 39); totals 295.2/297.4/298.2us (best-of-session 295.2
after a cool-down; the higher two were back-to-back thermally-inflated
runs).  WARNING: do
NOT replicate via gpsimd.partition_broadcast on BF16 [1,2048] -- it
corrupts memory (rel err 0.269 with zero biases); the proven gpsimd
broadcasts were FP32 [1,512].  FIXED: the ~432ns/head stall at the tail1
iteration (the only 7-matmul iteration: its extra pd MM rotated the
LDWEIGHTS weight-buffer-WAR phase) -- rebalancing the fold d-chunks to
(2,3,1,2,2,2,2,2) keeps every iteration at <=6 matmuls next to the pd;
totals 293.7/294.5us, the session's best two samples.  Also rejected: AllGather k/v dedup (72.6us cc-active per 1MB in
this harness); dma_start_transpose for v (neutral); warm-matmul fillers
in DMA-bound phase A (3 attempts, each +1..2us); fold-before-scores
order (neutral); psc bufs=3 (no PSUM left).
"""

import numpy as np
import ml_dtypes

B, S, D = 2, 2048, 2048
HQ, DH = 16, 128
SBLK = S // 4          # 512 query rows per core
N_CORES = 8
SCALE = 1.0 / float(np.sqrt(DH))

ND = D // 128          # 16 contraction chunks
NT = S // 128          # 16 key tiles
NQ = SBLK // 128       # 4 query row-tiles per core
NSH = NT // 2          # 8 pipeline steps per head

_cache = {}


def _round_fp32r(a):
    """Round fp32 to fp32r (1s+8e+11m) with round-to-nearest-even-ish."""
    b = np.ascontiguousarray(a, dtype=np.float32).view(np.uint32)
    bias = np.uint32(0x7FF) + ((b >> np.uint32(12)) & np.uint32(1))
    return ((b + bias) & np.uint32(0xFFFFF000)).view(np.float32)


def _build():
    from concourse import bacc, mybir, tile
    from concourse.masks import make_identity

    F32 = mybir.dt.float32
    F32R = mybir.dt.float32r
    BF16 = mybir.dt.bfloat16
    Exp = mybir.ActivationFunctionType.Exp
    Copy = mybir.ActivationFunctionType.Copy
    mult = mybir.AluOpType.mult
    add = mybir.AluOpType.add

    nc = bacc.Bacc("TRN2", target_bir_lowering=False, debug=False,
                   num_devices=N_CORES)

    xT = nc.dram_tensor("xT", [D, S], BF16, kind="ExternalInput").ap()
    xTq = nc.dram_tensor("xTq", [D, SBLK], BF16, kind="ExternalInput").ap()
    Wq = nc.dram_tensor("Wq", [D, D], BF16, kind="ExternalInput").ap()
    Wk = nc.dram_tensor("Wk", [D, DH], BF16, kind="ExternalInput").ap()
    Wv = nc.dram_tensor("Wv", [D, DH], BF16, kind="ExternalInput").ap()
    Wo = nc.dram_tensor("Wo", [D, D], BF16, kind="ExternalInput").ap()
    bqT_d = nc.dram_tensor("bqT", [1, D], F32R, kind="ExternalInput").ap()
    bkT_d = nc.dram_tensor("bkT", [1, DH], F32R, kind="ExternalInput").ap()
    bvT_d = nc.dram_tensor("bvT", [1, DH], F32R, kind="ExternalInput").ap()
    bo_d = nc.dram_tensor("bo", [1, D], F32R, kind="ExternalInput").ap()
    ones_d = nc.dram_tensor("ones", [128, 128], BF16, kind="ExternalInput").ap()
    ones_fd = nc.dram_tensor("onesf", [1, 128], F32R, kind="ExternalInput").ap()
    y = nc.dram_tensor("y", [SBLK, D], F32, kind="ExternalOutput").ap()

    with tile.TileContext(nc) as tc, nc.allow_low_precision(
        reason="bf16 matmul pipeline; verified against fp32 reference"
    ):
        with (
            tc.tile_pool(name="const", bufs=1) as cpool,
            tc.tile_pool(name="live", bufs=1) as lpool,      # kT, vT, v_nat, xq, qT
            tc.tile_pool(name="ot", bufs=HQ) as otpool,      # 16 head outputs
            tc.tile_pool(name="wq", bufs=4) as wqpool,       # Wq stream
            tc.tile_pool(name="pt", bufs=4) as ptpool,       # exp outputs
            tc.tile_pool(name="den", bufs=2) as dpool,       # denominator tree
            tc.tile_pool(name="rc", bufs=2) as rcpool,       # recip + broadcast
            tc.tile_pool(name="yp", bufs=3) as ypool,
        ):
            # ---- constants -------------------------------------------------
            ones = cpool.tile([128, 128], BF16)
            nc.sync.dma_start(out=ones[:, :], in_=ones_d[:, :])
            ones_col = ones[:, 0:1]
            ones_fr = cpool.tile([1, 128], F32R)
            nc.sync.dma_start(out=ones_fr[:, :], in_=ones_fd[:, :])
            bkT = cpool.tile([1, DH], F32R)
            nc.sync.dma_start(out=bkT[:, :], in_=bkT_d[:, :])
            bvT = cpool.tile([1, DH], F32R)
            nc.sync.dma_start(out=bvT[:, :], in_=bvT_d[:, :])
            bqT = cpool.tile([1, D], F32R)
            nc.sync.dma_start(out=bqT[:, :], in_=bqT_d[:, :])
            bo_row = cpool.tile([1, D], F32R)
            nc.sync.dma_start(out=bo_row[:, :], in_=bo_d[:, :])

            ones_row = cpool.tile([1, SBLK], BF16)
            nc.vector.memset(ones_row[:, :], 1.0)
            ones_big = cpool.tile([128, SBLK], BF16)
            nc.vector.memset(ones_big[:, :], 1.0)
            # biases replicated across partitions (values arrive /128, exact)
            # so bias matmuls use a full 128x128 stationary operand -- the
            # rank-1 1-partition lhsT broke PE weight-buffer pipelining
            # (~310+545ns of stalls per folded head).  Built below with PE
            # broadcast matmuls (the proven bo_b pattern; GpSimd
            # partition_broadcast corrupts bf16/[1,2048] operands).
            bqb = cpool.tile([128, D], BF16)
            bkb = cpool.tile([128, DH], BF16)
            bvb = cpool.tile([128, DH], BF16)
            warm_rhs = cpool.tile([128, SBLK], BF16)
            nc.vector.memset(warm_rhs[:, :], 0.0)
            warm_lhs = cpool.tile([128, 128], BF16)
            nc.vector.memset(warm_lhs[:, :], 0.0)
            ident = cpool.tile([128, 128], BF16)
            make_identity(nc, ident[:, :])
            bo_b = cpool.tile([128, D], F32)

            warm = cpool.tile([1, 1], BF16)
            nc.scalar.activation(warm[:, :], ones[0:1, 0:1], Exp, scale=1.0)

            kT = lpool.tile([128, S], BF16)
            vT = lpool.tile([128, S], BF16)
            v_nat = lpool.tile([128, NT, DH], BF16)
            xq = lpool.tile([128, ND, SBLK], BF16)
            qT_all = lpool.tile([128, HQ, SBLK], BF16)

            # q-projection PSUM bank allocated BEFORE phase A's pool so the
            # bootstrap q-projections never wait on the pool-wide barrier
            # against phase A's eight PSUM drains.
            pqp = tc.alloc_tile_pool(name="pq", bufs=1, space="PSUM")

            def q_proj(h, wq_t):
                pq = pqp.tile([128, SBLK], F32, tag="pq", name=f"pq{h}")
                for d in range(ND):
                    nc.tensor.matmul(
                        pq[:, :], lhsT=wq_t[:, d, :], rhs=xq[:, d, :],
                        start=(d == 0), stop=False,
                    )
                nc.tensor.matmul(
                    pq[:, :], lhsT=bqb[:, h * 128:(h + 1) * 128],
                    rhs=ones_big[:, :], start=False, stop=True,
                )
                nc.scalar.activation(qT_all[:, h, :], pq[:, :], Copy)

            # ---- phase A: k/v projections over the full sequence -----------
            with tc.tile_pool(name="pha", bufs=1) as apool:
                wk_all = apool.tile([128, ND, DH], BF16)
                nc.sync.dma_start(
                    out=wk_all[:, :, :],
                    in_=Wk.rearrange("(n p) d -> p n d", p=128),
                )
                wv_all = apool.tile([128, ND, DH], BF16)
                nc.sync.dma_start(
                    out=wv_all[:, :, :],
                    in_=Wv.rearrange("(n p) d -> p n d", p=128),
                )
                # xT chunks stream in consumption order; xq and the first
                # q-weight tiles are interleaved late enough not to delay the
                # matmul stream (PE has buffered chunks by then) but early
                # enough to be resident when the bootstrap needs them.
                wq_tiles = {}

                def wq_dma(h):
                    wq_tiles[h] = wqpool.tile([128, ND, 128], BF16, tag="wq",
                                              name=f"wq{h}")
                    nc.sync.dma_start(
                        out=wq_tiles[h][:, :, :],
                        in_=Wq[:, h * 128:(h + 1) * 128].rearrange(
                            "(n p) m -> p n m", p=128
                        ),
                    )

                # xT chunks keep DMA-queue priority (they pace phase A); the
                # xq quarters and first wq tiles slot in late, where the PE
                # has buffered chunks, and land just before the bootstrap
                # needs them.
                xT_all = apool.tile([128, ND, S], BF16)
                xq_quarter = {9: 0, 10: 1, 12: 2, 13: 3}
                for d in range(ND):
                    nc.sync.dma_start(
                        out=xT_all[:, d, :], in_=xT[d * 128:(d + 1) * 128, :]
                    )
                    if d in xq_quarter:
                        q = xq_quarter[d]
                        nc.sync.dma_start(
                            out=xq[:, 4 * q:4 * (q + 1), :],
                            in_=xTq[512 * q:512 * (q + 1), :].rearrange(
                                "(n p) s -> p n s", p=128
                            ),
                        )
                    if d == 11:
                        wq_dma(0)
                    elif d == 15:
                        wq_dma(1)
                        wq_dma(2)

                with tc.tile_pool(name="pacc", bufs=1, space="PSUM") as pacc:
                    psum_k = pacc.tile([128, S], F32, tag="pk")
                    psum_v = pacc.tile([128, 1536], F32, tag="pv")

                    # PE warm-up in the (pre-allocated) q bank: keep the
                    # array busy while DMAs land so the HAM clock gate opens
                    # before the real stream begins.  No DMA dependencies.
                    warm_t = pqp.tile([128, SBLK], F32, tag="pq",
                                      name="warmpq")
                    for w in range(16):
                        nc.tensor.matmul(
                            warm_t[:, :],
                            lhsT=warm_lhs[:, :], rhs=warm_rhs[:, :],
                            start=True, stop=True,
                        )
                    # replicate biases across partitions (DMA-bound window,
                    # the PE/DVE round-trips here are free)
                    for j in range(D // 512):
                        nc.tensor.matmul(
                            warm_t[:, :], lhsT=ones_fr[0:1, :],
                            rhs=bqT[0:1, j * 512:(j + 1) * 512],
                            start=True, stop=True,
                        )
                        nc.vector.tensor_copy(
                            bqb[:, j * 512:(j + 1) * 512], warm_t[:, :]
                        )
                    nc.tensor.matmul(
                        warm_t[:, 0:DH], lhsT=ones_fr[0:1, :],
                        rhs=bkT[0:1, :], start=True, stop=True,
                    )
                    nc.vector.tensor_copy(bkb[:, :], warm_t[:, 0:DH])
                    nc.tensor.matmul(
                        warm_t[:, 0:DH], lhsT=ones_fr[0:1, :],
                        rhs=bvT[0:1, :], start=True, stop=True,
                    )
                    nc.vector.tensor_copy(bvb[:, :], warm_t[:, 0:DH])

                    for d in range(ND):
                        for nb in range(4):
                            sl = slice(nb * 512, (nb + 1) * 512)
                            nc.tensor.matmul(
                                psum_k[:, sl],
                                lhsT=wk_all[:, d, :],
                                rhs=xT_all[:, d, sl],
                                start=(d == 0), stop=False,
                            )
                        for nb in range(3):
                            sl = slice(nb * 512, (nb + 1) * 512)
                            nc.tensor.matmul(
                                psum_v[:, sl],
                                lhsT=wv_all[:, d, :],
                                rhs=xT_all[:, d, sl],
                                start=(d == 0), stop=False,
                            )
                        if d == 14:
                            # fill the DMA-starved tail of the chunk stream
                            q_proj(0, wq_tiles[0])
                    # rank-1 bias updates close each accumulation group
                    for nb in range(4):
                        sl = slice(nb * 512, (nb + 1) * 512)
                        nc.tensor.matmul(
                            psum_k[:, sl], lhsT=bkb[:, :],
                            rhs=ones_big[:, :], start=False, stop=True,
                        )
                    for nb in range(3):
                        sl = slice(nb * 512, (nb + 1) * 512)
                        nc.tensor.matmul(
                            psum_v[:, sl], lhsT=bvb[:, :],
                            rhs=ones_big[:, :], start=False, stop=True,
                        )
                    # drains split across DVE + ScalarE; kT block 0 first so
                    # its bank is free for v block 3's second pass below.
                    nc.vector.tensor_copy(kT[:, 0:512], psum_k[:, 0:512])
                    nc.scalar.activation(vT[:, 0:512], psum_v[:, 0:512], Copy)
                    nc.vector.tensor_copy(vT[:, 512:1024], psum_v[:, 512:1024])
                    nc.scalar.activation(vT[:, 1024:1536], psum_v[:, 1024:1536],
                                         Copy)
                    nc.vector.tensor_copy(kT[:, 512:1024], psum_k[:, 512:1024])
                    nc.scalar.activation(kT[:, 1024:1536], psum_k[:, 1024:1536],
                                         Copy)
                    nc.vector.tensor_copy(kT[:, 1536:2048], psum_k[:, 1536:2048])
                    # q1 covers the kT block-0 drain latency on the PE
                    q_proj(1, wq_tiles[1])
                    # v block 3 (keys 1536-2048): second pass through the
                    # freed k bank.  Pure PE work; the chunk data is SBUF
                    # resident, so this rides the DMA-bound region for free.
                    for d in range(ND):
                        nc.tensor.matmul(
                            psum_k[:, 0:512], lhsT=wv_all[:, d, :],
                            rhs=xT_all[:, d, 1536:2048],
                            start=(d == 0), stop=False,
                        )
                    nc.tensor.matmul(
                        psum_k[:, 0:512], lhsT=bvb[:, :],
                        rhs=ones_big[:, :], start=False, stop=True,
                    )
                    nc.scalar.activation(vT[:, 1536:2048], psum_k[:, 0:512],
                                         Copy)

            # ---- bootstrap: v transposes + bo broadcast --------------------
            with tc.tile_pool(name="ptr", bufs=1, space="PSUM") as ptrp:
                # copies split across VectorE/ScalarE: B1's first scores wait
                # on these PSUM banks, and a serial DVE backlog here was the
                # 3.5us stall (plus HAM re-throttle) at the B1 handoff.
                for t in range(NT):
                    ptr = ptrp.tile([128, 128], BF16, tag="tr", bufs=2)
                    nc.tensor.transpose(
                        ptr[:, :], vT[:, t * 128:(t + 1) * 128], ident[:, :]
                    )
                    if t % 2 == 0:
                        nc.vector.tensor_copy(v_nat[:, t, :], ptr[:, :])
                    else:
                        nc.scalar.activation(v_nat[:, t, :], ptr[:, :], Copy)
                # bias broadcast for phase C: bo_b = ones(128) x bo_row
                for nb in range(D // 512):
                    sl = slice(nb * 512, (nb + 1) * 512)
                    pbo = ptrp.tile([128, 512], F32, tag="bo", bufs=2)
                    nc.tensor.matmul(
                        pbo[:, :], lhsT=ones_fr[0:1, :], rhs=bo_row[0:1, sl],
                        start=True, stop=True,
                    )
                    if nb % 2 == 0:
                        nc.scalar.activation(bo_b[:, sl], pbo[:, :], Copy)
                    else:
                        nc.vector.tensor_copy(bo_b[:, sl], pbo[:, :])

            # ---- phase B1': fused attention + q-projection pipeline --------
            outT_list = [None] * HQ
            NS = HQ * NSH
            with (
                tc.tile_pool(name="psc", bufs=2, space="PSUM") as pscp,
                tc.tile_pool(name="po", bufs=2, space="PSUM") as pop,
                tc.tile_pool(name="aux", bufs=1, space="PSUM") as auxp,
            ):
                pT_t, P_t, Bt_t, Ct_t, root_t = {}, {}, {}, {}, {}
                po_t, rc_t = {}, {}

                def emit_scores(s):
                    h, tp = divmod(s, NSH)
                    psc = pscp.tile([128, 2 * SBLK], F32, tag="sc")
                    for half in range(2):
                        t = tp * 2 + half
                        nc.tensor.matmul(
                            psc[:, half * SBLK:(half + 1) * SBLK],
                            lhsT=kT[:, t * 128:(t + 1) * 128],
                            rhs=qT_all[:, h, :],
                            start=True, stop=True,
                        )
                    return psc

                def emit_exp(s, psc):
                    h, tp = divmod(s, NSH)
                    pT = ptpool.tile([128, 2 * SBLK], BF16, tag="pT")
                    nc.scalar.activation(pT[:, :], psc[:, :], Exp, scale=SCALE)
                    pT_t[s] = pT
                    # denominator: pairwise add + bf16 tree on DVE
                    P = dpool.tile([128, SBLK], BF16, tag="P", name=f"P{tp}")
                    nc.vector.tensor_tensor(
                        P[:, :], pT[:, 0:SBLK], pT[:, SBLK:2 * SBLK], add
                    )
                    P_t[tp] = P
                    if tp % 2 == 1:
                        Bt = dpool.tile([128, SBLK], BF16, tag="B",
                                        name=f"B{tp // 2}")
                        nc.vector.tensor_tensor(
                            Bt[:, :], P_t.pop(tp - 1)[:, :], P_t.pop(tp)[:, :],
                            add,
                        )
                        Bt_t[tp // 2] = Bt
                    if tp in (3, 7):
                        Ct = dpool.tile([128, SBLK], BF16, tag="C",
                                        name=f"C{tp // 4}")
                        nc.vector.tensor_tensor(
                            Ct[:, :], Bt_t.pop(tp // 2 - 1)[:, :],
                            Bt_t.pop(tp // 2)[:, :], add,
                        )
                        Ct_t[tp // 4] = Ct
                    if tp == 7:
                        root = dpool.tile([128, SBLK], BF16, tag="root",
                                          name=f"root{h}")
                        nc.vector.tensor_tensor(
                            root[:, :], Ct_t.pop(0)[:, :], Ct_t.pop(1)[:, :],
                            add,
                        )
                        root_t[h] = root

                def emit_pv(s):
                    h, tp = divmod(s, NSH)
                    if tp == 0:
                        po_t[h] = pop.tile([128, SBLK], F32, tag="po",
                                           name=f"po{h}")
                    pT = pT_t.pop(s)
                    for half in range(2):
                        t = tp * 2 + half
                        nc.tensor.matmul(
                            po_t[h][:, :],
                            lhsT=v_nat[:, t, :],
                            rhs=pT[:, half * SBLK:(half + 1) * SBLK],
                            start=(t == 0), stop=(t == NT - 1),
                        )

                def emit_tail1(h):
                    # partition-reduce the tree root + reciprocal.  Runs 3
                    # steps after the head's last scores so the DVE tree root
                    # is long done when the PE matmul wants it.
                    # lhsT is the full 128x128 ones tile: every output
                    # partition gets the same column sum, i.e. the reduce IS
                    # the broadcast, at the same N=512 matmul cost — and the
                    # 128-col LDWEIGHTS pipelines like every other weight
                    # load (a 1-col load breaks the weight-buffer rhythm).
                    pd = auxp.tile([128, SBLK], F32, tag="aux", name=f"pd{h}")
                    nc.tensor.matmul(
                        pd[:, :], lhsT=ones[:, :], rhs=root_t.pop(h)[:, :],
                        start=True, stop=True,
                    )
                    rb = rcpool.tile([128, SBLK], F32, tag="rb", name=f"rb{h}")
                    nc.vector.reciprocal_approx_fast(rb[:, :], pd[:, :])
                    rc_t[h] = rb

                def emit_tail2(h):
                    # normalize; 3 steps after tail1 so the PE never waits on
                    # the DVE->GpSimd round-trip.
                    outT = otpool.tile([128, SBLK], BF16, tag="ot",
                                       name=f"ot{h}")
                    nc.vector.tensor_tensor(
                        outT[:, :], po_t.pop(h)[:, :], rc_t.pop(h)[:, :], mult
                    )
                    outT_list[h] = outT

                def emit_fold(s):
                    # q-projection for head h+2, 2 matmuls per step
                    h, tp = divmod(s, NSH)
                    hf = h + 2
                    if hf >= HQ:
                        return
                    if tp == 0:
                        # queue DMA for the next folded head's weights
                        hn = hf + 1
                        if hn < HQ:
                            wq_tiles[hn] = wqpool.tile(
                                [128, ND, 128], BF16, tag="wq", name=f"wq{hn}"
                            )
                            nc.sync.dma_start(
                                out=wq_tiles[hn][:, :, :],
                                in_=Wq[:, hn * 128:(hn + 1) * 128].rearrange(
                                    "(n p) m -> p n m", p=128
                                ),
                            )
                        # stream this head's slice of Wo for phase C
                        if h < 8:
                            for hh in range(2):
                                hw = h * 2 + hh
                                for db in range(D // 512):
                                    dsl = slice(db * 512, (db + 1) * 512)
                                    wt = wopool.tile(
                                        [128, 512], BF16, tag="wo",
                                        name=f"wo{db}_{hw}"
                                    )
                                    nc.sync.dma_start(
                                        out=wt[:, :],
                                        in_=Wo[hw * 128:(hw + 1) * 128, dsl],
                                    )
                                    wo_tiles[db, hw] = wt
                        fold_pq[hf] = pqp.tile([128, SBLK], F32, tag="pq",
                                               name=f"pqf{hf}")
                    pq = fold_pq[hf]
                    # tp2 carries the previous head's pd reduce matmul; give
                    # it only 1 fold MM (3 in tp1) so no iteration exceeds 6
                    # matmuls -- the 7-MM tail1 iteration rotated the
                    # LDWEIGHTS weight-buffer-WAR phase into a ~432ns stall.
                    counts = (2, 3, 1, 2, 2, 2, 2, 2)
                    d0 = sum(counts[:tp])
                    for d in range(d0, d0 + counts[tp]):
                        nc.tensor.matmul(
                            pq[:, :], lhsT=wq_tiles[hf][:, d, :],
                            rhs=xq[:, d, :], start=(d == 0), stop=False,
                        )
                    if tp == NSH - 1:
                        nc.tensor.matmul(
                            pq[:, :], lhsT=bqb[:, hf * 128:(hf + 1) * 128],
                            rhs=ones_big[:, :], start=False, stop=True,
                        )
                        nc.scalar.activation(qT_all[:, hf, :], pq[:, :], Copy)

                # Wo prefetch pool opens after phase-A SBUF is released
                wopool = tc.alloc_tile_pool(name="wo", bufs=64)
                wo_tiles = {}
                fold_pq = {}
                cpre = {}

                psc_t = {}
                for s in range(NS + 6):
                    if s < NS:
                        psc_t[s] = emit_scores(s)
                        emit_fold(s)
                    if s == 2:
                        # dependency-free filler: the pipeline has no p@v
                        # work yet and scores(2) waits on exp(0), so the PE
                        # would idle here (and HAM would re-throttle the
                        # clock).  Aux bank is unused until head 0's tail.
                        wa = auxp.tile([128, SBLK], F32, tag="aux",
                                       name="warmaux")
                        for w in range(6):
                            nc.tensor.matmul(
                                wa[:, :], lhsT=warm_lhs[:, :],
                                rhs=warm_rhs[:, :], start=True, stop=True,
                            )
                    if 0 <= s - 1 < NS:
                        emit_exp(s - 1, psc_t.pop(s - 1))
                    if 0 <= s - 2 < NS:
                        emit_pv(s - 2)
                    if 0 <= s - 3 < NS and (s - 3) % NSH == NSH - 1:
                        emit_tail1((s - 3) // NSH)
                    if 0 <= s - 6 < NS and (s - 6) % NSH == NSH - 1:
                        emit_tail2((s - 6) // NSH)
                    # heads 14/15 have no fold work, so those steps idle
                    # against exp: pre-execute phase C's first chain there,
                    # 1 matmul per step -- head hh's outT is normalized at
                    # step 8*hh+13 <= 112+hh, exactly in time.
                    if 112 <= s <= 126:
                        hhp = s - 112
                        if hhp == 0:
                            cpre["py"] = pqp.tile([128, 512], F32, tag="pq",
                                                  name="py0")
                        nc.tensor.matmul(
                            cpre["py"][:, :],
                            lhsT=outT_list[hhp][:, 0:128],
                            rhs=wo_tiles[0, hhp][:, :],
                            start=(hhp == 0), stop=False,
                        )

            # ---- phase C: output projection y = out @ Wo + bo --------------
            # finish the pre-executed first chain (head 15 normalizes after
            # the pipeline drains), then the remaining 15 chains
            nc.tensor.matmul(
                cpre["py"][:, :], lhsT=outT_list[15][:, 0:128],
                rhs=wo_tiles[0, 15][:, :], start=False, stop=True,
            )
            y_sb0 = ypool.tile([128, 512], F32, tag="y", name="ysb0")
            nc.vector.tensor_tensor(
                y_sb0[:, :], cpre["py"][:, :], bo_b[:, 0:512], add
            )
            nc.sync.dma_start(out=y[0:128, 0:512], in_=y_sb0[:, :])
            with tc.tile_pool(name="py", bufs=3, space="PSUM") as pyp:
                for db in range(D // 512):
                    dsl = slice(db * 512, (db + 1) * 512)
                    for st in range(NQ):
                        if db == 0 and st == 0:
                            continue
                        py = pyp.tile([128, 512], F32, tag="py")
                        for hh in range(HQ):
                            nc.tensor.matmul(
                                py[:, :],
                                lhsT=outT_list[hh][:, st * 128:(st + 1) * 128],
                                rhs=wo_tiles[db, hh][:, :],
                                start=(hh == 0), stop=(hh == HQ - 1),
                            )
                        y_sb = ypool.tile([128, 512], F32, tag="y")
                        nc.vector.tensor_tensor(
                            y_sb[:, :], py[:, :], bo_b[:, dsl], add
                        )
                        nc.sync.dma_start(
                            out=y[st * 128:(st + 1) * 128, dsl], in_=y_sb[:, :]
                        )

            wopool.release()
            pqp.release()

    nc.compile()
    return nc


def _get_nc():
    if "nc" not in _cache:
        _cache["nc"] = _build()
    return _cache["nc"]


def _prepare_in_maps(x, Wq, bq, Wk, bk, Wv, bv, Wo, bo):
    bf = ml_dtypes.bfloat16
    x = np.asarray(x, dtype=np.float32)
    bqT = (_round_fp32r(bq) / np.float32(128))[None, :]
    bkT = (_round_fp32r(bk) / np.float32(128))[None, :]
    bvT = (_round_fp32r(bv) / np.float32(128))[None, :]
    bo = _round_fp32r(bo)[None, :]
    Wq_b = np.asarray(Wq, np.float32).astype(bf)
    Wk_b = np.asarray(Wk, np.float32).astype(bf)
    Wv_b = np.asarray(Wv, np.float32).astype(bf)
    Wo_b = np.asarray(Wo, np.float32).astype(bf)
    ones = np.ones((128, 128), bf)
    onesf = np.ones((1, 128), np.float32)

    xT = [np.ascontiguousarray(x[g].T).astype(bf) for g in range(B)]
    in_maps = []
    for c in range(N_CORES):
        g, blk = divmod(c, 4)
        s0 = blk * SBLK
        in_maps.append({
            "xT": xT[g],
            "xTq": np.ascontiguousarray(xT[g][:, s0:s0 + SBLK]),
            "Wq": Wq_b, "Wk": Wk_b, "Wv": Wv_b, "Wo": Wo_b,
            "bqT": bqT, "bkT": bkT, "bvT": bvT, "bo": bo,
            "ones": ones, "onesf": onesf,
        })
    return in_maps


def _assemble(results):
    out = np.empty((B, S, D), dtype=np.float32)
    for c in range(N_CORES):
        g, blk = divmod(c, 4)
        out[g, blk * SBLK:(blk + 1) * SBLK, :] = results[c]["y"]
    return out


def kernel(x, Wq, bq, Wk, bk, Wv, bv, Wo, bo):
    from concourse.bass_utils import run_bass_kernel_spmd

    in_maps = _prepare_in_maps(x, Wq, bq, Wk, bk, Wv, bv, Wo, bo)
    nc = _get_nc()
    res = run_bass_kernel_spmd(nc, in_maps, core_ids=list(range(N_CORES)))
    return _assemble(res.results)
